# revision 16
# baseline (speedup 1.0000x reference)
"""MultiHeadCrossAttention kernel for 8 Trainium2 NeuronCores.

Problem (hardcoded): B=4, Sx=Sy=1024, DIM=1024, H=16, Dh=64, fp32.
  Q = x@W_Qx.T+b_Qx ; K = cat(x@W_Kx.T+b_Kx, y@W_Ky.T+b_Ky) per head
  V = cat(x@W_Vx.T+b_Vx, y@W_Vy.T+b_Vy) ; out = softmax(QK^T/8)V @ W_out.T + b_out

Sharding: core c -> (batch b = c//2, head-group g = c%2 of 8 heads).
Each core computes its batch's attention for its 8 heads plus the partial
out-projection over its 512 features; host sums the two partials per batch
and adds b_out (the "all-reduce after to_out", done in the gather).

Device layout choices (all matmuls natural, zero on-device transposes):
 - activations pre-transposed on host: xT/yT [dim, seq]
 - Q/K projections in transposed domain [feat, seq]  (bias = per-partition)
 - V in natural domain [seq, feat] with host-broadcast bias, plus a ones
   column per head -> AV matmul row 64 yields the softmax denominator
 - scoresT [k, q] via lhsT=KT (d=64 contraction; head pairs row-pack the PE)
 - exp on ACT only (no max subtraction: |scores| <~ 3), normalize via
   PE-broadcast reciprocal, out-projection in transposed domain [m, s]
 - float32r everywhere on the PE: full rate at N=512, ~5e-5 rel err
"""

import os
import sys

os.environ.setdefault("MYCRO_LOCAL_CACHE", "1")
if "/opt/trn_rl_repo" not in sys.path:
    sys.path.insert(0, "/opt/trn_rl_repo")

import ml_dtypes
import numpy as np

import concourse.bass as bass
import concourse.mybir as mybir
import concourse.tile as tile
from concourse import bass_utils
from concourse.bass_utils import run_bass_kernel_spmd

FP32 = mybir.dt.float32
FP32R = mybir.dt.float32r
BF16 = mybir.dt.bfloat16

DIM = 1024
H = 16          # total heads
HG = 8          # heads per core (head-group)
DH = 64
S = 1024        # Sx = Sy
FS = 512        # feature slice per core (HG * DH)
NCORES = 8

# ---------------------------------------------------------------------------
# harness patches (this snapshot's Tile emits >1 wait per instruction in a
# few places; HW instructions hold one wait)
# ---------------------------------------------------------------------------

def _patched_drain_and_barrier(self, tick_clock, wait_clock):
    from bass_rust import ScopedClock

    nc = self.nc
    drain_inst = nc.sync.drain()
    wait_clock.add_sem_waits(
        drain_inst.ins, ScopedClock({None: tick_clock.global_clock})
    )
    si = drain_inst.ins.sync_info
    waits = list(si.on_wait)
    if len(waits) > 1:
        del si.on_wait[1:]
        for w in waits[1:]:
            nop = nc.sync.nop(nofuse=True, hint="drain_wait_spill")
            if nop.ins.sync_info is None:
                nop.ins.sync_info = mybir.SyncInfo(on_wait=[], on_update=[])
            nop.ins.sync_info.on_wait.append(w)

    nc.all_engine_barrier()
    assert self.sems is not None
    popped = nc._tile_sem_poison_stack.pop()
    assert popped is self._sem_poison
    nc.clear_and_free_semaphores(list(self.sems.allocated().values()))
    nc.all_engine_barrier()


def _spill_excess_waits(nc):
    n = 0
    for fn in nc.m.functions:
        for bb in fn.blocks:
            new_insts = []
            for inst in bb.instructions:
                si = getattr(inst, "sync_info", None)
                cap = 2 if isinstance(inst, mybir.InstEventSemaphore) else 1
                if si is not None and si.on_wait and len(si.on_wait) > cap:
                    extras = list(si.on_wait[cap:])
                    del si.on_wait[cap:]
                    for w in extras:
                        new_insts.append(
                            mybir.InstNoOp(
                                name=f"wspill-{nc.next_id()}",
                                engine=inst.engine,
                                ins=[],
                                outs=[],
                                sync_info=mybir.SyncInfo(on_wait=[w], on_update=[]),
                            )
                        )
                        n += 1
                new_insts.append(inst)
            bb.instructions[:] = new_insts
    return n


tile.TileContext._drain_and_barrier = _patched_drain_and_barrier

if os.environ.get("ENABLE_LDW_OPT") == "1":
    _orig_run_command = bass_utils.run_command

    def _run_command_ldw(argv, **kwargs):
        if isinstance(argv, list):
            argv = ["--enable-ldw-opt=true" if a == "--enable-ldw-opt=false" else a
                    for a in argv]
        return _orig_run_command(argv, **kwargs)

    bass_utils.run_command = _run_command_ldw
bass_utils.upload_artifacts = lambda tmpdir: tmpdir  # no S3 in container


def _register_ntff_hook():
    """Best-effort: enables trace=True runs (used by test harness only)."""
    try:
        from antenv.axon_hooks import set_axon_ntff_profile_hook
        sys.path.insert(0, "/root/.axon_site")
        from trn_agent_boot.trn_boot import _ntff_profile_via_ctypes

        set_axon_ntff_profile_hook(
            _ntff_profile_via_ctypes("/opt/axon/libaxon_pjrt.so")
        )
    except Exception:
        pass


# ---------------------------------------------------------------------------
# device program (identical on all 8 cores; per-core data differs)
# ---------------------------------------------------------------------------

def _build_program():
    nc = bass.Bass()

    xT = nc.declare_dram_parameter("xT", [DIM, S], BF16, isOutput=False)
    yT = nc.declare_dram_parameter("yT", [DIM, S], BF16, isOutput=False)
    wq = nc.declare_dram_parameter("wq", [DIM, FS], BF16, isOutput=False)
    wkx = nc.declare_dram_parameter("wkx", [DIM, FS], BF16, isOutput=False)
    wky = nc.declare_dram_parameter("wky", [DIM, FS], BF16, isOutput=False)
    wvx = nc.declare_dram_parameter("wvx", [DIM, FS], BF16, isOutput=False)
    wvy = nc.declare_dram_parameter("wvy", [DIM, FS], BF16, isOutput=False)
    wo = nc.declare_dram_parameter("wo", [FS, DIM], BF16, isOutput=False)
    bq = nc.declare_dram_parameter("bq", [128, 4], FP32, isOutput=False)
    bkx = nc.declare_dram_parameter("bkx", [128, 4], FP32, isOutput=False)
    bky = nc.declare_dram_parameter("bky", [128, 4], FP32, isOutput=False)
    bvx_bc = nc.declare_dram_parameter("bvx_bc", [128, FS], FP32, isOutput=False)
    bvy_bc = nc.declare_dram_parameter("bvy_bc", [128, FS], FP32, isOutput=False)
    outT = nc.declare_dram_parameter("outT", [DIM, S], FP32, isOutput=True)

    EXP = mybir.ActivationFunctionType.Exp

    with tile.TileContext(nc) as tc:
        import contextlib

        with contextlib.ExitStack() as ctx:
            big = ctx.enter_context(tc.tile_pool(name="big", bufs=16))
            wpool = ctx.enter_context(tc.tile_pool(name="wpool", bufs=8))
            qkv = ctx.enter_context(tc.tile_pool(name="qkv", bufs=12))
            vpool = ctx.enter_context(tc.tile_pool(name="vpool", bufs=16))
            ppool = ctx.enter_context(tc.tile_pool(name="ppool", bufs=5))
            opool = ctx.enter_context(tc.tile_pool(name="opool", bufs=2))
            spool = ctx.enter_context(tc.tile_pool(name="spool", bufs=4))
            cpool = ctx.enter_context(tc.tile_pool(name="cpool", bufs=1))
            dpool = ctx.enter_context(tc.tile_pool(name="dpool", bufs=8, space="DRAM"))
            mm_ps = ctx.enter_context(tc.tile_pool(name="mm_ps", bufs=2, space="PSUM"))
            ot_ps = ctx.enter_context(tc.tile_pool(name="ot_ps", bufs=4, space="PSUM"))

            # ---- constants ----
            ones_f32 = cpool.tile([128, 64], FP32, tag="ones_f32")
            nc.vector.memset(ones_f32[:, :], 1.0)
            bq_sb = cpool.tile([128, 4], FP32, tag="bq")
            bkx_sb = cpool.tile([128, 4], FP32, tag="bkx")
            bky_sb = cpool.tile([128, 4], FP32, tag="bky")
            bvx_sb = cpool.tile([128, FS], FP32, tag="bvx")
            bvy_sb = cpool.tile([128, FS], FP32, tag="bvy")
            nc.sync.dma_start(out=bq_sb, in_=bq[:, :])
            nc.sync.dma_start(out=bkx_sb, in_=bkx[:, :])
            nc.sync.dma_start(out=bky_sb, in_=bky[:, :])
            nc.sync.dma_start(out=bvx_sb, in_=bvx_bc[:, :])
            nc.sync.dma_start(out=bvy_sb, in_=bvy_bc[:, :])

            # ---- load activations ----
            xt = []
            yt = []
            for i in range(8):
                t = big.tile([128, S], BF16, tag="big", name=f"xt{i}")
                nc.sync.dma_start(out=t, in_=xT[i * 128:(i + 1) * 128, :])
                xt.append(t)
            for i in range(8):
                t = big.tile([128, S], BF16, tag="big", name=f"yt{i}")
                nc.sync.dma_start(out=t, in_=yT[i * 128:(i + 1) * 128, :])
                yt.append(t)

            # ---- V projections (natural domain, bias + ones column) ----
            V = [vpool.tile([128, HG, DH + 1], BF16, tag="v", name=f"V{i}") for i in range(16)]
            for src_is_y in (False, True):
                w_dram = wvy if src_is_y else wvx
                act = yt if src_is_y else xt
                bias_sb = bvy_sb if src_is_y else bvx_sb
                base = 8 if src_is_y else 0
                w_sb = []
                for ct in range(8):
                    t = wpool.tile([128, FS], BF16, tag="w", name=f"wv{ct}")
                    nc.sync.dma_start(
                        out=t, in_=w_dram[ct * 128:(ct + 1) * 128, :]
                    )
                    w_sb.append(t)
                for sg in range(4):  # two s-tiles per psum group
                    ps = mm_ps.tile([128, 1024], FP32, tag="mm", name="vps")
                    for ct in range(8):
                        for half in range(2):
                            st = 2 * sg + half
                            nc.tensor.matmul(
                                ps[:, half * 512:(half + 1) * 512],
                                xt[ct][:, st * 128:(st + 1) * 128] if not src_is_y
                                else yt[ct][:, st * 128:(st + 1) * 128],
                                w_sb[ct][:, :],
                                start=(ct == 0),
                                stop=(ct == 7),
                            )
                    for half in range(2):
                        st = 2 * sg + half
                        vt = V[base + st]
                        nc.vector.tensor_add(
                            out=vt[:, :, 0:DH],
                            in0=ps[:, half * 512:(half + 1) * 512].rearrange(
                                "p (h d) -> p h d", h=HG),
                            in1=bias_sb[:, :].rearrange("p (h d) -> p h d", h=HG),
                        )
                        nc.vector.tensor_copy(
                            out=vt[:, :, DH:DH + 1],
                            in_=ones_f32[:, 0:HG].rearrange("p (h o) -> p h o", o=1),
                        )

            # ---- Q/K projections (transposed domain [feat, seq]) ----
            QT = [qkv.tile([128, S], BF16, tag="qkv", name=f"QT{i}") for i in range(4)]
            KxT = [qkv.tile([128, S], BF16, tag="qkv", name=f"KxT{i}") for i in range(4)]
            KyT = [qkv.tile([128, S], BF16, tag="qkv", name=f"KyT{i}") for i in range(4)]

            for w_dram, act, bias_sb, dst in (
                (wq, xt, bq_sb, QT),
                (wkx, xt, bkx_sb, KxT),
                (wky, yt, bky_sb, KyT),
            ):
                w_sb = []
                for ct in range(8):
                    t = wpool.tile([128, FS], BF16, tag="w", name=f"wp{ct}")
                    nc.sync.dma_start(
                        out=t, in_=w_dram[ct * 128:(ct + 1) * 128, :]
                    )
                    w_sb.append(t)
                for ft in range(4):
                    ps = mm_ps.tile([128, 1024], FP32, tag="mm", name="qps")
                    for ct in range(8):
                        for half in range(2):
                            nc.tensor.matmul(
                                ps[:, half * 512:(half + 1) * 512],
                                w_sb[ct][:, ft * 128:(ft + 1) * 128],
                                act[ct][:, half * 512:(half + 1) * 512],
                                start=(ct == 0),
                                stop=(ct == 7),
                            )
                    nc.vector.tensor_scalar_add(
                        out=dst[ft][:, :],
                        in0=ps[:, :],
                        scalar1=bias_sb[:, ft:ft + 1],
                    )

            # ---- attention (head pairs row-pack; both q-tiles share one
            #      psum tile so exp runs once per (kt, head)) ----
            oT = [big.tile([128, S], BF16, tag="big", name=f"oT{i}") for i in range(4)]

            def emit_finalize(t, o_sb, recips):
                for qt in range(2):
                    for hh in range(2):
                        i = 2 * qt + hh
                        rd = recips[i]
                        bc_sb = spool.tile([DH, 512], FP32, tag="bc", name="bc_sb")
                        rd_bcast = bass.AP(
                            tensor=rd.tensor, offset=rd.offset,
                            ap=[[0, DH]] + [list(a) for a in rd.ap[1:]],
                        )
                        nc.gpsimd.dma_start(out=bc_sb[:, :], in_=rd_bcast)
                        nc.vector.tensor_mul(
                            out=oT[t][hh * 64:hh * 64 + DH, qt * 512:(qt + 1) * 512],
                            in0=o_sb[i][0:DH, :],
                            in1=bc_sb[:, :],
                        )

            pending = None
            for t in range(4):  # heads 2t, 2t+1
                o_ps = [ot_ps.tile([128, 512], FP32, tag="ot", name=f"ops{i}")
                        for i in range(4)]  # index 2*qt+hh
                prev = None
                for kt in range(16):
                    KT = KxT[t] if kt < 8 else KyT[t]
                    ks = (kt % 8) * 128
                    p_sb = []
                    for hh in range(2):
                        sc = mm_ps.tile([128, 1024], FP32, tag="mm", name="sc")
                        for qt in range(2):
                            nc.tensor.matmul(
                                sc[:, qt * 512:(qt + 1) * 512],
                                KT[hh * 64:(hh + 1) * 64, ks:ks + 128],
                                QT[t][hh * 64:(hh + 1) * 64, qt * 512:(qt + 1) * 512],
                                start=True,
                                stop=True,
                            )
                        p = ppool.tile([128, 1024], BF16, tag="p", name="p")
                        nc.scalar.activation(out=p[:, :], in_=sc[:, :], func=EXP)
                        p_sb.append(p)
                    if prev is not None:
                        for qt in range(2):
                            for hh in range(2):
                                nc.tensor.matmul(
                                    o_ps[2 * qt + hh][0:DH + 1, :],
                                    V[kt - 1][:, 2 * t + hh, :],
                                    prev[hh][:, qt * 512:(qt + 1) * 512],
                                    start=(kt == 1),
                                    stop=False,
                                )
                    prev = p_sb
                for qt in range(2):
                    for hh in range(2):
                        nc.tensor.matmul(
                            o_ps[2 * qt + hh][0:DH + 1, :],
                            V[15][:, 2 * t + hh, :],
                            prev[hh][:, qt * 512:(qt + 1) * 512],
                            start=False,
                            stop=True,
                        )
                if pending is not None:
                    emit_finalize(*pending)
                o_sb = []
                recips = []
                for i in range(4):
                    ob = spool.tile([DH + 1, 512], FP32, tag="osb", name="osb")
                    nc.vector.tensor_copy(out=ob[:, :], in_=o_ps[i][0:DH + 1, :])
                    o_sb.append(ob)
                for i in range(4):
                    rf = spool.tile([1, 512], FP32, tag="recipf", name="rf")
                    nc.vector.reciprocal(out=rf[:, :], in_=o_sb[i][DH:DH + 1, :])
                    rd = dpool.tile([1, 512], FP32, name="rd")
                    nc.gpsimd.dma_start(out=rd[:, :], in_=rf[:, :])
                    recips.append(rd)
                pending = (t, o_sb, recips)
            emit_finalize(*pending)

            # ---- out-projection (transposed domain [m, s]) ----
            wo_sb = []
            for ft in range(4):
                t = big.tile([128, S], BF16, tag="big", name=f"wo{ft}")
                nc.sync.dma_start(
                    out=t, in_=wo[ft * 128:(ft + 1) * 128, :]
                )
                wo_sb.append(t)
            for mt in range(8):
                ps = mm_ps.tile([128, 1024], FP32, tag="mm", name="obs")
                for ft in range(4):
                    for half in range(2):
                        nc.tensor.matmul(
                            ps[:, half * 512:(half + 1) * 512],
                            wo_sb[ft][:, mt * 128:(mt + 1) * 128],
                            oT[ft][:, half * 512:(half + 1) * 512],
                            start=(ft == 0),
                            stop=(ft == 3),
                        )
                osb = opool.tile([128, 1024], FP32, tag="osb2", name="osb2")
                nc.vector.tensor_copy(out=osb[:, :], in_=ps[:, :])
                nc.sync.dma_start(
                    out=outT[mt * 128:(mt + 1) * 128, :],
                    in_=osb[:, :],
                )

    _spill_excess_waits(nc)
    return nc


_NC = None


def _get_program():
    global _NC
    if _NC is None:
        _NC = _build_program()
    return _NC


# ---------------------------------------------------------------------------
# host wrapper
# ---------------------------------------------------------------------------

def _prep_in_maps(x, y, W_Kx, b_Kx, W_Qx, b_Qx, W_Vx, b_Vx, W_Ky, b_Ky,
                  W_Vy, b_Vy, W_out, b_out):
    f32 = np.float32
    bf16 = ml_dtypes.bfloat16
    in_maps = []
    for c in range(NCORES):
        b = c // 2
        g = c % 2
        gs = slice(FS * g, FS * (g + 1))
        m = {
            "xT": np.ascontiguousarray(np.asarray(x[b], f32).T).astype(bf16),
            "yT": np.ascontiguousarray(np.asarray(y[b], f32).T).astype(bf16),
            "wq": np.ascontiguousarray((np.asarray(W_Qx, f32)[gs, :] / 8.0).T).astype(bf16),
            "wkx": np.ascontiguousarray(np.asarray(W_Kx, f32)[gs, :].T).astype(bf16),
            "wky": np.ascontiguousarray(np.asarray(W_Ky, f32)[gs, :].T).astype(bf16),
            "wvx": np.ascontiguousarray(np.asarray(W_Vx, f32)[gs, :].T).astype(bf16),
            "wvy": np.ascontiguousarray(np.asarray(W_Vy, f32)[gs, :].T).astype(bf16),
            "wo": np.ascontiguousarray(np.asarray(W_out, f32)[:, gs].T).astype(bf16),
            "bq": np.ascontiguousarray(
                (np.asarray(b_Qx, f32)[gs] / 8.0).reshape(4, 128).T),
            "bkx": np.ascontiguousarray(np.asarray(b_Kx, f32)[gs].reshape(4, 128).T),
            "bky": np.ascontiguousarray(np.asarray(b_Ky, f32)[gs].reshape(4, 128).T),
            "bvx_bc": np.ascontiguousarray(
                np.broadcast_to(np.asarray(b_Vx, f32)[gs], (128, FS))),
            "bvy_bc": np.ascontiguousarray(
                np.broadcast_to(np.asarray(b_Vy, f32)[gs], (128, FS))),
        }
        in_maps.append(m)
    return in_maps


def _assemble(results, b_out):
    B = 4
    out = np.empty((B, S, DIM), np.float32)
    bo = np.asarray(b_out, np.float32)
    for b in range(B):
        acc = results[2 * b]["outT"] + results[2 * b + 1]["outT"]
        out[b] = acc.T + bo
    return out


def kernel(**inputs):
    nc = _get_program()
    in_maps = _prep_in_maps(**inputs)
    res = run_bass_kernel_spmd(nc, in_maps, core_ids=list(range(NCORES)))
    return _assemble(res.results, inputs["b_out"])


def kernel_traced(trace_cores=None, **inputs):
    """Same as kernel() but returns (out, BassKernelResults) with NTFF trace."""
    _register_ntff_hook()
    nc = _get_program()
    in_maps = _prep_in_maps(**inputs)
    res = run_bass_kernel_spmd(
        nc, in_maps, core_ids=list(range(NCORES)), trace=True,
        trace_cores=trace_cores or [0],
    )
    return _assemble(res.results, inputs["b_out"]), res


# revision 17
# speedup vs baseline: 1.3806x; 1.3806x over previous
"""MultiHeadCrossAttention kernel for 8 Trainium2 NeuronCores.

Problem (hardcoded): B=4, Sx=Sy=1024, DIM=1024, H=16, Dh=64, fp32.
  Q = x@W_Qx.T+b_Qx ; K = cat(x@W_Kx.T+b_Kx, y@W_Ky.T+b_Ky) per head
  V = cat(x@W_Vx.T+b_Vx, y@W_Vy.T+b_Vy) ; out = softmax(QK^T/8)V @ W_out.T + b_out

Sharding: core c -> (batch b = c//2, head-group g = c%2 of 8 heads).
Each core computes its batch's attention for its 8 heads plus the partial
out-projection over its 512 features; host sums the two partials per batch
and adds b_out (the "all-reduce after to_out", done in the gather).

Device layout choices (all matmuls natural, zero on-device transposes):
 - activations pre-transposed on host: xT/yT [dim, seq]
 - Q/K projections in transposed domain [feat, seq]  (bias = per-partition)
 - V in natural domain [seq, feat] with host-broadcast bias, plus a ones
   column per head -> AV matmul row 64 yields the softmax denominator
 - scoresT [k, q] via lhsT=KT (d=64 contraction; head pairs row-pack the PE)
 - exp on ACT only (no max subtraction: |scores| <~ 3), normalize via
   PE-broadcast reciprocal, out-projection in transposed domain [m, s]
 - float32r everywhere on the PE: full rate at N=512, ~5e-5 rel err
"""

import os
import sys

os.environ.setdefault("MYCRO_LOCAL_CACHE", "1")
if "/opt/trn_rl_repo" not in sys.path:
    sys.path.insert(0, "/opt/trn_rl_repo")

import ml_dtypes
import numpy as np

import concourse.bass as bass
import concourse.mybir as mybir
import concourse.tile as tile
from concourse import bass_utils
from concourse.bass_utils import run_bass_kernel_spmd

FP32 = mybir.dt.float32
FP32R = mybir.dt.float32r
BF16 = mybir.dt.bfloat16

DIM = 1024
H = 16          # total heads
HG = 8          # heads per core (head-group)
DH = 64
S = 1024        # Sx = Sy
FS = 512        # feature slice per core (HG * DH)
NCORES = 8

# ---------------------------------------------------------------------------
# harness patches (this snapshot's Tile emits >1 wait per instruction in a
# few places; HW instructions hold one wait)
# ---------------------------------------------------------------------------

def _patched_drain_and_barrier(self, tick_clock, wait_clock):
    from bass_rust import ScopedClock

    nc = self.nc
    drain_inst = nc.sync.drain()
    wait_clock.add_sem_waits(
        drain_inst.ins, ScopedClock({None: tick_clock.global_clock})
    )
    si = drain_inst.ins.sync_info
    waits = list(si.on_wait)
    if len(waits) > 1:
        del si.on_wait[1:]
        for w in waits[1:]:
            nop = nc.sync.nop(nofuse=True, hint="drain_wait_spill")
            if nop.ins.sync_info is None:
                nop.ins.sync_info = mybir.SyncInfo(on_wait=[], on_update=[])
            nop.ins.sync_info.on_wait.append(w)

    nc.all_engine_barrier()
    assert self.sems is not None
    popped = nc._tile_sem_poison_stack.pop()
    assert popped is self._sem_poison
    nc.clear_and_free_semaphores(list(self.sems.allocated().values()))
    nc.all_engine_barrier()


def _spill_excess_waits(nc):
    n = 0
    for fn in nc.m.functions:
        for bb in fn.blocks:
            new_insts = []
            for inst in bb.instructions:
                si = getattr(inst, "sync_info", None)
                cap = 2 if isinstance(inst, mybir.InstEventSemaphore) else 1
                if si is not None and si.on_wait and len(si.on_wait) > cap:
                    extras = list(si.on_wait[cap:])
                    del si.on_wait[cap:]
                    for w in extras:
                        new_insts.append(
                            mybir.InstNoOp(
                                name=f"wspill-{nc.next_id()}",
                                engine=inst.engine,
                                ins=[],
                                outs=[],
                                sync_info=mybir.SyncInfo(on_wait=[w], on_update=[]),
                            )
                        )
                        n += 1
                new_insts.append(inst)
            bb.instructions[:] = new_insts
    return n


tile.TileContext._drain_and_barrier = _patched_drain_and_barrier

if os.environ.get("ENABLE_LDW_OPT") == "1":
    _orig_run_command = bass_utils.run_command

    def _run_command_ldw(argv, **kwargs):
        if isinstance(argv, list):
            argv = ["--enable-ldw-opt=true" if a == "--enable-ldw-opt=false" else a
                    for a in argv]
        return _orig_run_command(argv, **kwargs)

    bass_utils.run_command = _run_command_ldw
bass_utils.upload_artifacts = lambda tmpdir: tmpdir  # no S3 in container


def _register_ntff_hook():
    """Best-effort: enables trace=True runs (used by test harness only)."""
    try:
        from antenv.axon_hooks import set_axon_ntff_profile_hook
        sys.path.insert(0, "/root/.axon_site")
        from trn_agent_boot.trn_boot import _ntff_profile_via_ctypes

        set_axon_ntff_profile_hook(
            _ntff_profile_via_ctypes("/opt/axon/libaxon_pjrt.so")
        )
    except Exception:
        pass


# ---------------------------------------------------------------------------
# device program (identical on all 8 cores; per-core data differs)
# ---------------------------------------------------------------------------

def _build_program():
    nc = bass.Bass()

    xT = nc.declare_dram_parameter("xT", [DIM, S], BF16, isOutput=False)
    yT = nc.declare_dram_parameter("yT", [DIM, S], BF16, isOutput=False)
    wq = nc.declare_dram_parameter("wq", [DIM, FS], BF16, isOutput=False)
    wkx = nc.declare_dram_parameter("wkx", [DIM, FS], BF16, isOutput=False)
    wky = nc.declare_dram_parameter("wky", [DIM, FS], BF16, isOutput=False)
    wvx = nc.declare_dram_parameter("wvx", [DIM, FS], BF16, isOutput=False)
    wvy = nc.declare_dram_parameter("wvy", [DIM, FS], BF16, isOutput=False)
    wo = nc.declare_dram_parameter("wo", [FS, DIM], BF16, isOutput=False)
    bq = nc.declare_dram_parameter("bq", [128, 4], FP32, isOutput=False)
    bkx = nc.declare_dram_parameter("bkx", [128, 4], FP32, isOutput=False)
    bky = nc.declare_dram_parameter("bky", [128, 4], FP32, isOutput=False)
    bvx_bc = nc.declare_dram_parameter("bvx_bc", [128, FS], FP32, isOutput=False)
    bvy_bc = nc.declare_dram_parameter("bvy_bc", [128, FS], FP32, isOutput=False)
    outT = nc.declare_dram_parameter("outT", [DIM, S], FP32, isOutput=True)

    EXP = mybir.ActivationFunctionType.Exp

    with tile.TileContext(nc) as tc:
        import contextlib

        with contextlib.ExitStack() as ctx:
            big = ctx.enter_context(tc.tile_pool(name="big", bufs=16))
            wpool = ctx.enter_context(tc.tile_pool(name="wpool", bufs=8))
            qkv = ctx.enter_context(tc.tile_pool(name="qkv", bufs=12))
            vpool = ctx.enter_context(tc.tile_pool(name="vpool", bufs=16))
            ppool = ctx.enter_context(tc.tile_pool(name="ppool", bufs=5))
            opool = ctx.enter_context(tc.tile_pool(name="opool", bufs=2))
            spool = ctx.enter_context(tc.tile_pool(name="spool", bufs=4))
            cpool = ctx.enter_context(tc.tile_pool(name="cpool", bufs=1))
            dpool = ctx.enter_context(tc.tile_pool(name="dpool", bufs=8, space="DRAM"))
            mm_ps = ctx.enter_context(tc.tile_pool(name="mm_ps", bufs=2, space="PSUM"))
            ot_ps = ctx.enter_context(tc.tile_pool(name="ot_ps", bufs=4, space="PSUM"))

            # ---- constants ----
            ones_f32 = cpool.tile([128, 64], FP32, tag="ones_f32")
            nc.vector.memset(ones_f32[:, :], 1.0)
            bq_sb = cpool.tile([128, 4], FP32, tag="bq")
            bkx_sb = cpool.tile([128, 4], FP32, tag="bkx")
            bky_sb = cpool.tile([128, 4], FP32, tag="bky")
            bvx_sb = cpool.tile([128, FS], FP32, tag="bvx")
            bvy_sb = cpool.tile([128, FS], FP32, tag="bvy")
            nc.sync.dma_start(out=bq_sb, in_=bq[:, :])
            nc.sync.dma_start(out=bkx_sb, in_=bkx[:, :])
            nc.sync.dma_start(out=bky_sb, in_=bky[:, :])
            nc.sync.dma_start(out=bvx_sb, in_=bvx_bc[:, :])
            nc.sync.dma_start(out=bvy_sb, in_=bvy_bc[:, :])

            # ---- load activations ----
            xt = []
            yt = []
            for i in range(8):
                t = big.tile([128, S], BF16, tag="big", name=f"xt{i}")
                nc.sync.dma_start(out=t, in_=xT[i * 128:(i + 1) * 128, :])
                xt.append(t)
            for i in range(8):
                t = big.tile([128, S], BF16, tag="big", name=f"yt{i}")
                nc.sync.dma_start(out=t, in_=yT[i * 128:(i + 1) * 128, :])
                yt.append(t)

            # ---- V projections (natural domain, bias + ones column) ----
            V = [vpool.tile([128, HG, DH + 1], BF16, tag="v", name=f"V{i}") for i in range(16)]
            for src_is_y in (False, True):
                w_dram = wvy if src_is_y else wvx
                act = yt if src_is_y else xt
                bias_sb = bvy_sb if src_is_y else bvx_sb
                base = 8 if src_is_y else 0
                w_sb = []
                for ct in range(8):
                    t = wpool.tile([128, FS], BF16, tag="w", name=f"wv{ct}")
                    nc.sync.dma_start(
                        out=t, in_=w_dram[ct * 128:(ct + 1) * 128, :]
                    )
                    w_sb.append(t)
                for sg in range(4):  # two s-tiles per psum group
                    ps = mm_ps.tile([128, 1024], FP32, tag="mm", name="vps")
                    for ct in range(8):
                        for half in range(2):
                            st = 2 * sg + half
                            nc.tensor.matmul(
                                ps[:, half * 512:(half + 1) * 512],
                                xt[ct][:, st * 128:(st + 1) * 128] if not src_is_y
                                else yt[ct][:, st * 128:(st + 1) * 128],
                                w_sb[ct][:, :],
                                start=(ct == 0),
                                stop=(ct == 7),
                            )
                    for half in range(2):
                        st = 2 * sg + half
                        vt = V[base + st]
                        nc.vector.tensor_add(
                            out=vt[:, :, 0:DH],
                            in0=ps[:, half * 512:(half + 1) * 512].rearrange(
                                "p (h d) -> p h d", h=HG),
                            in1=bias_sb[:, :].rearrange("p (h d) -> p h d", h=HG),
                        )
                        nc.vector.tensor_copy(
                            out=vt[:, :, DH:DH + 1],
                            in_=ones_f32[:, 0:HG].rearrange("p (h o) -> p h o", o=1),
                        )

            # ---- Q/K projections (transposed domain [feat, seq]) ----
            QT = [qkv.tile([128, S], BF16, tag="qkv", name=f"QT{i}") for i in range(4)]
            KxT = [qkv.tile([128, S], BF16, tag="qkv", name=f"KxT{i}") for i in range(4)]
            KyT = [qkv.tile([128, S], BF16, tag="qkv", name=f"KyT{i}") for i in range(4)]

            for w_dram, act, bias_sb, dst in (
                (wq, xt, bq_sb, QT),
                (wkx, xt, bkx_sb, KxT),
                (wky, yt, bky_sb, KyT),
            ):
                w_sb = []
                for ct in range(8):
                    t = wpool.tile([128, FS], BF16, tag="w", name=f"wp{ct}")
                    nc.sync.dma_start(
                        out=t, in_=w_dram[ct * 128:(ct + 1) * 128, :]
                    )
                    w_sb.append(t)
                for ft in range(4):
                    ps = mm_ps.tile([128, 1024], FP32, tag="mm", name="qps")
                    for ct in range(8):
                        for half in range(2):
                            nc.tensor.matmul(
                                ps[:, half * 512:(half + 1) * 512],
                                w_sb[ct][:, ft * 128:(ft + 1) * 128],
                                act[ct][:, half * 512:(half + 1) * 512],
                                start=(ct == 0),
                                stop=(ct == 7),
                            )
                    nc.vector.tensor_scalar_add(
                        out=dst[ft][:, :],
                        in0=ps[:, :],
                        scalar1=bias_sb[:, ft:ft + 1],
                    )

            # ---- attention (head pairs row-pack; both q-tiles share one
            #      psum tile so exp runs once per (kt, head)) ----
            oT = [big.tile([128, S], BF16, tag="big", name=f"oT{i}") for i in range(4)]

            def emit_finalize(t, qt, o_sb, recips):
                for hh in range(2):
                        i = hh
                        rd = recips[i]
                        bc_sb = spool.tile([DH, 512], FP32, tag="bc", name="bc_sb")
                        rd_bcast = bass.AP(
                            tensor=rd.tensor, offset=rd.offset,
                            ap=[[0, DH]] + [list(a) for a in rd.ap[1:]],
                        )
                        nc.gpsimd.dma_start(out=bc_sb[:, :], in_=rd_bcast)
                        nc.vector.tensor_mul(
                            out=oT[t][hh * 64:hh * 64 + DH, qt * 512:(qt + 1) * 512],
                            in0=o_sb[i][0:DH, :],
                            in1=bc_sb[:, :],
                        )

            pending = None
            for t in range(4):  # heads 2t, 2t+1
                for qt in range(2):
                    o_ps = [ot_ps.tile([128, 512], FP32, tag="ot", name=f"ops{i}")
                            for i in range(2)]  # per head of the pair
                    prev = None
                    for kt in range(16):
                        KT = KxT[t] if kt < 8 else KyT[t]
                        ks = (kt % 8) * 128
                        sc = mm_ps.tile([128, 1024], FP32, tag="mm", name="sc")
                        for hh in range(2):
                            nc.tensor.matmul(
                                sc[:, hh * 512:(hh + 1) * 512],
                                KT[hh * 64:(hh + 1) * 64, ks:ks + 128],
                                QT[t][hh * 64:(hh + 1) * 64, qt * 512:(qt + 1) * 512],
                                start=True,
                                stop=True,
                            )
                        p2 = ppool.tile([128, 1024], BF16, tag="p", name="p")
                        nc.scalar.activation(out=p2[:, :], in_=sc[:, :], func=EXP)
                        if prev is not None:
                            for hh in range(2):
                                nc.tensor.matmul(
                                    o_ps[hh][0:DH + 1, :],
                                    V[kt - 1][:, 2 * t + hh, :],
                                    prev[:, hh * 512:(hh + 1) * 512],
                                    start=(kt == 1),
                                    stop=False,
                                )
                        prev = p2
                    for hh in range(2):
                        nc.tensor.matmul(
                            o_ps[hh][0:DH + 1, :],
                            V[15][:, 2 * t + hh, :],
                            prev[:, hh * 512:(hh + 1) * 512],
                            start=False,
                            stop=True,
                        )
                    if pending is not None:
                        emit_finalize(*pending)
                    o_sb = []
                    recips = []
                    for i in range(2):
                        ob = spool.tile([DH + 1, 512], FP32, tag="osb", name="osb")
                        nc.vector.tensor_copy(out=ob[:, :], in_=o_ps[i][0:DH + 1, :])
                        o_sb.append(ob)
                    for i in range(2):
                        rf = spool.tile([1, 512], FP32, tag="recipf", name="rf")
                        nc.vector.reciprocal(out=rf[:, :], in_=o_sb[i][DH:DH + 1, :])
                        rd = dpool.tile([1, 512], FP32, name="rd")
                        nc.gpsimd.dma_start(out=rd[:, :], in_=rf[:, :])
                        recips.append(rd)
                    pending = (t, qt, o_sb, recips)
            emit_finalize(*pending)

            # ---- out-projection (transposed domain [m, s]) ----
            wo_sb = []
            for ft in range(4):
                t = big.tile([128, S], BF16, tag="big", name=f"wo{ft}")
                nc.sync.dma_start(
                    out=t, in_=wo[ft * 128:(ft + 1) * 128, :]
                )
                wo_sb.append(t)
            for mt in range(8):
                ps = mm_ps.tile([128, 1024], FP32, tag="mm", name="obs")
                for ft in range(4):
                    for half in range(2):
                        nc.tensor.matmul(
                            ps[:, half * 512:(half + 1) * 512],
                            wo_sb[ft][:, mt * 128:(mt + 1) * 128],
                            oT[ft][:, half * 512:(half + 1) * 512],
                            start=(ft == 0),
                            stop=(ft == 3),
                        )
                osb = opool.tile([128, 1024], FP32, tag="osb2", name="osb2")
                nc.vector.tensor_copy(out=osb[:, :], in_=ps[:, :])
                nc.sync.dma_start(
                    out=outT[mt * 128:(mt + 1) * 128, :],
                    in_=osb[:, :],
                )

    _spill_excess_waits(nc)
    return nc


_NC = None


def _get_program():
    global _NC
    if _NC is None:
        _NC = _build_program()
    return _NC


# ---------------------------------------------------------------------------
# host wrapper
# ---------------------------------------------------------------------------

def _prep_in_maps(x, y, W_Kx, b_Kx, W_Qx, b_Qx, W_Vx, b_Vx, W_Ky, b_Ky,
                  W_Vy, b_Vy, W_out, b_out):
    f32 = np.float32
    bf16 = ml_dtypes.bfloat16
    in_maps = []
    for c in range(NCORES):
        b = c // 2
        g = c % 2
        gs = slice(FS * g, FS * (g + 1))
        m = {
            "xT": np.ascontiguousarray(np.asarray(x[b], f32).T).astype(bf16),
            "yT": np.ascontiguousarray(np.asarray(y[b], f32).T).astype(bf16),
            "wq": np.ascontiguousarray((np.asarray(W_Qx, f32)[gs, :] / 8.0).T).astype(bf16),
            "wkx": np.ascontiguousarray(np.asarray(W_Kx, f32)[gs, :].T).astype(bf16),
            "wky": np.ascontiguousarray(np.asarray(W_Ky, f32)[gs, :].T).astype(bf16),
            "wvx": np.ascontiguousarray(np.asarray(W_Vx, f32)[gs, :].T).astype(bf16),
            "wvy": np.ascontiguousarray(np.asarray(W_Vy, f32)[gs, :].T).astype(bf16),
            "wo": np.ascontiguousarray(np.asarray(W_out, f32)[:, gs].T).astype(bf16),
            "bq": np.ascontiguousarray(
                (np.asarray(b_Qx, f32)[gs] / 8.0).reshape(4, 128).T),
            "bkx": np.ascontiguousarray(np.asarray(b_Kx, f32)[gs].reshape(4, 128).T),
            "bky": np.ascontiguousarray(np.asarray(b_Ky, f32)[gs].reshape(4, 128).T),
            "bvx_bc": np.ascontiguousarray(
                np.broadcast_to(np.asarray(b_Vx, f32)[gs], (128, FS))),
            "bvy_bc": np.ascontiguousarray(
                np.broadcast_to(np.asarray(b_Vy, f32)[gs], (128, FS))),
        }
        in_maps.append(m)
    return in_maps


def _assemble(results, b_out):
    B = 4
    out = np.empty((B, S, DIM), np.float32)
    bo = np.asarray(b_out, np.float32)
    for b in range(B):
        acc = results[2 * b]["outT"] + results[2 * b + 1]["outT"]
        out[b] = acc.T + bo
    return out


def kernel(**inputs):
    nc = _get_program()
    in_maps = _prep_in_maps(**inputs)
    res = run_bass_kernel_spmd(nc, in_maps, core_ids=list(range(NCORES)))
    return _assemble(res.results, inputs["b_out"])


def kernel_traced(trace_cores=None, **inputs):
    """Same as kernel() but returns (out, BassKernelResults) with NTFF trace."""
    _register_ntff_hook()
    nc = _get_program()
    in_maps = _prep_in_maps(**inputs)
    res = run_bass_kernel_spmd(
        nc, in_maps, core_ids=list(range(NCORES)), trace=True,
        trace_cores=trace_cores or [0],
    )
    return _assemble(res.results, inputs["b_out"]), res


# revision 18
# speedup vs baseline: 1.4042x; 1.0171x over previous
"""MultiHeadCrossAttention kernel for 8 Trainium2 NeuronCores.

Problem (hardcoded): B=4, Sx=Sy=1024, DIM=1024, H=16, Dh=64, fp32.
  Q = x@W_Qx.T+b_Qx ; K = cat(x@W_Kx.T+b_Kx, y@W_Ky.T+b_Ky) per head
  V = cat(x@W_Vx.T+b_Vx, y@W_Vy.T+b_Vy) ; out = softmax(QK^T/8)V @ W_out.T + b_out

Sharding: core c -> (batch b = c//2, head-group g = c%2 of 8 heads).
Each core computes its batch's attention for its 8 heads plus the partial
out-projection over its 512 features; host sums the two partials per batch
and adds b_out (the "all-reduce after to_out", done in the gather).

Device layout choices (all matmuls natural, zero on-device transposes):
 - activations pre-transposed on host: xT/yT [dim, seq]
 - Q/K projections in transposed domain [feat, seq]  (bias = per-partition)
 - V in natural domain [seq, feat] with host-broadcast bias, plus a ones
   column per head -> AV matmul row 64 yields the softmax denominator
 - scoresT [k, q] via lhsT=KT (d=64 contraction; head pairs row-pack the PE)
 - exp on ACT only (no max subtraction: |scores| <~ 3), normalize via
   PE-broadcast reciprocal, out-projection in transposed domain [m, s]
 - float32r everywhere on the PE: full rate at N=512, ~5e-5 rel err
"""

import os
import sys

os.environ.setdefault("MYCRO_LOCAL_CACHE", "1")
if "/opt/trn_rl_repo" not in sys.path:
    sys.path.insert(0, "/opt/trn_rl_repo")

import ml_dtypes
import numpy as np

import concourse.bass as bass
import concourse.mybir as mybir
import concourse.tile as tile
from concourse import bass_utils
from concourse.bass_utils import run_bass_kernel_spmd

FP32 = mybir.dt.float32
FP32R = mybir.dt.float32r
BF16 = mybir.dt.bfloat16

DIM = 1024
H = 16          # total heads
HG = 8          # heads per core (head-group)
DH = 64
S = 1024        # Sx = Sy
FS = 512        # feature slice per core (HG * DH)
NCORES = 8

# ---------------------------------------------------------------------------
# harness patches (this snapshot's Tile emits >1 wait per instruction in a
# few places; HW instructions hold one wait)
# ---------------------------------------------------------------------------

def _patched_drain_and_barrier(self, tick_clock, wait_clock):
    from bass_rust import ScopedClock

    nc = self.nc
    drain_inst = nc.sync.drain()
    wait_clock.add_sem_waits(
        drain_inst.ins, ScopedClock({None: tick_clock.global_clock})
    )
    si = drain_inst.ins.sync_info
    waits = list(si.on_wait)
    if len(waits) > 1:
        del si.on_wait[1:]
        for w in waits[1:]:
            nop = nc.sync.nop(nofuse=True, hint="drain_wait_spill")
            if nop.ins.sync_info is None:
                nop.ins.sync_info = mybir.SyncInfo(on_wait=[], on_update=[])
            nop.ins.sync_info.on_wait.append(w)

    nc.all_engine_barrier()
    assert self.sems is not None
    popped = nc._tile_sem_poison_stack.pop()
    assert popped is self._sem_poison
    nc.clear_and_free_semaphores(list(self.sems.allocated().values()))
    nc.all_engine_barrier()


def _spill_excess_waits(nc):
    n = 0
    for fn in nc.m.functions:
        for bb in fn.blocks:
            new_insts = []
            for inst in bb.instructions:
                si = getattr(inst, "sync_info", None)
                cap = 2 if isinstance(inst, mybir.InstEventSemaphore) else 1
                if si is not None and si.on_wait and len(si.on_wait) > cap:
                    extras = list(si.on_wait[cap:])
                    del si.on_wait[cap:]
                    for w in extras:
                        new_insts.append(
                            mybir.InstNoOp(
                                name=f"wspill-{nc.next_id()}",
                                engine=inst.engine,
                                ins=[],
                                outs=[],
                                sync_info=mybir.SyncInfo(on_wait=[w], on_update=[]),
                            )
                        )
                        n += 1
                new_insts.append(inst)
            bb.instructions[:] = new_insts
    return n


tile.TileContext._drain_and_barrier = _patched_drain_and_barrier

if os.environ.get("ENABLE_LDW_OPT") == "1":
    _orig_run_command = bass_utils.run_command

    def _run_command_ldw(argv, **kwargs):
        if isinstance(argv, list):
            argv = ["--enable-ldw-opt=true" if a == "--enable-ldw-opt=false" else a
                    for a in argv]
        return _orig_run_command(argv, **kwargs)

    bass_utils.run_command = _run_command_ldw
bass_utils.upload_artifacts = lambda tmpdir: tmpdir  # no S3 in container


def _register_ntff_hook():
    """Best-effort: enables trace=True runs (used by test harness only)."""
    try:
        from antenv.axon_hooks import set_axon_ntff_profile_hook
        sys.path.insert(0, "/root/.axon_site")
        from trn_agent_boot.trn_boot import _ntff_profile_via_ctypes

        set_axon_ntff_profile_hook(
            _ntff_profile_via_ctypes("/opt/axon/libaxon_pjrt.so")
        )
    except Exception:
        pass


# ---------------------------------------------------------------------------
# device program (identical on all 8 cores; per-core data differs)
# ---------------------------------------------------------------------------

def _build_program():
    nc = bass.Bass()

    xT = nc.declare_dram_parameter("xT", [DIM, S], BF16, isOutput=False)
    yT = nc.declare_dram_parameter("yT", [DIM, S], BF16, isOutput=False)
    wq = nc.declare_dram_parameter("wq", [DIM, FS], BF16, isOutput=False)
    wkx = nc.declare_dram_parameter("wkx", [DIM, FS], BF16, isOutput=False)
    wky = nc.declare_dram_parameter("wky", [DIM, FS], BF16, isOutput=False)
    wvx = nc.declare_dram_parameter("wvx", [DIM, FS], BF16, isOutput=False)
    wvy = nc.declare_dram_parameter("wvy", [DIM, FS], BF16, isOutput=False)
    wo = nc.declare_dram_parameter("wo", [FS, DIM], BF16, isOutput=False)
    bq = nc.declare_dram_parameter("bq", [128, 4], FP32, isOutput=False)
    bkx = nc.declare_dram_parameter("bkx", [128, 4], FP32, isOutput=False)
    bky = nc.declare_dram_parameter("bky", [128, 4], FP32, isOutput=False)
    bvx_bc = nc.declare_dram_parameter("bvx_bc", [128, FS], FP32, isOutput=False)
    bvy_bc = nc.declare_dram_parameter("bvy_bc", [128, FS], FP32, isOutput=False)
    outT = nc.declare_dram_parameter("outT", [DIM, S], FP32, isOutput=True)

    EXP = mybir.ActivationFunctionType.Exp

    with tile.TileContext(nc) as tc:
        import contextlib

        with contextlib.ExitStack() as ctx:
            big = ctx.enter_context(tc.tile_pool(name="big", bufs=16))
            wpool = ctx.enter_context(tc.tile_pool(name="wpool", bufs=8))
            qkv = ctx.enter_context(tc.tile_pool(name="qkv", bufs=12))
            vpool = ctx.enter_context(tc.tile_pool(name="vpool", bufs=16))
            ppool = ctx.enter_context(tc.tile_pool(name="ppool", bufs=5))
            opool = ctx.enter_context(tc.tile_pool(name="opool", bufs=2))
            spool = ctx.enter_context(tc.tile_pool(name="spool", bufs=4))
            cpool = ctx.enter_context(tc.tile_pool(name="cpool", bufs=1))
            dpool = ctx.enter_context(tc.tile_pool(name="dpool", bufs=8, space="DRAM"))
            mm_ps = ctx.enter_context(tc.tile_pool(name="mm_ps", bufs=2, space="PSUM"))
            ot_ps = ctx.enter_context(tc.tile_pool(name="ot_ps", bufs=2, space="PSUM"))

            # ---- constants ----
            ones_f32 = cpool.tile([128, 64], FP32, tag="ones_f32")
            nc.vector.memset(ones_f32[:, :], 1.0)
            bq_sb = cpool.tile([128, 4], FP32, tag="bq")
            bkx_sb = cpool.tile([128, 4], FP32, tag="bkx")
            bky_sb = cpool.tile([128, 4], FP32, tag="bky")
            bvx_sb = cpool.tile([128, FS], FP32, tag="bvx")
            bvy_sb = cpool.tile([128, FS], FP32, tag="bvy")
            nc.sync.dma_start(out=bq_sb, in_=bq[:, :])
            nc.sync.dma_start(out=bkx_sb, in_=bkx[:, :])
            nc.sync.dma_start(out=bky_sb, in_=bky[:, :])
            nc.sync.dma_start(out=bvx_sb, in_=bvx_bc[:, :])
            nc.sync.dma_start(out=bvy_sb, in_=bvy_bc[:, :])

            # ---- load activations ----
            xt = []
            for i in range(8):
                t = big.tile([128, S], BF16, tag="big", name=f"xt{i}")
                nc.sync.dma_start(out=t, in_=xT[i * 128:(i + 1) * 128, :])
                xt.append(t)
            yt = []

            # ---- V projections (natural domain, bias + ones column) ----
            V = [vpool.tile([128, HG, DH + 1], BF16, tag="v", name=f"V{i}") for i in range(16)]
            for src_is_y in (False, True):
                w_dram = wvy if src_is_y else wvx
                bias_sb = bvy_sb if src_is_y else bvx_sb
                base = 8 if src_is_y else 0
                if src_is_y and not yt:
                    for i in range(8):
                        ty = big.tile([128, S], BF16, tag="big", name=f"yt{i}")
                        nc.sync.dma_start(out=ty, in_=yT[i * 128:(i + 1) * 128, :])
                        yt.append(ty)
                act = yt if src_is_y else xt
                w_sb = []
                for ct in range(8):
                    t = wpool.tile([128, FS], BF16, tag="w", name=f"wv{ct}")
                    nc.sync.dma_start(
                        out=t, in_=w_dram[ct * 128:(ct + 1) * 128, :]
                    )
                    w_sb.append(t)
                for sg in range(4):  # two s-tiles per psum group
                    ps = mm_ps.tile([128, 1024], FP32, tag="mm", name="vps")
                    for ct in range(8):
                        for half in range(2):
                            st = 2 * sg + half
                            nc.tensor.matmul(
                                ps[:, half * 512:(half + 1) * 512],
                                xt[ct][:, st * 128:(st + 1) * 128] if not src_is_y
                                else yt[ct][:, st * 128:(st + 1) * 128],
                                w_sb[ct][:, :],
                                start=(ct == 0),
                                stop=(ct == 7),
                            )
                    for half in range(2):
                        st = 2 * sg + half
                        vt = V[base + st]
                        nc.vector.tensor_add(
                            out=vt[:, :, 0:DH],
                            in0=ps[:, half * 512:(half + 1) * 512].rearrange(
                                "p (h d) -> p h d", h=HG),
                            in1=bias_sb[:, :].rearrange("p (h d) -> p h d", h=HG),
                        )
                        nc.vector.tensor_copy(
                            out=vt[:, :, DH:DH + 1],
                            in_=ones_f32[:, 0:HG].rearrange("p (h o) -> p h o", o=1),
                        )

            # ---- Q/K projections (transposed domain [feat, seq]) ----
            QT = [qkv.tile([128, S], BF16, tag="qkv", name=f"QT{i}") for i in range(4)]
            KxT = [qkv.tile([128, S], BF16, tag="qkv", name=f"KxT{i}") for i in range(4)]
            KyT = [qkv.tile([128, S], BF16, tag="qkv", name=f"KyT{i}") for i in range(4)]

            for w_dram, act, bias_sb, dst in (
                (wq, xt, bq_sb, QT),
                (wkx, xt, bkx_sb, KxT),
                (wky, yt, bky_sb, KyT),
            ):
                w_sb = []
                for ct in range(8):
                    t = wpool.tile([128, FS], BF16, tag="w", name=f"wp{ct}")
                    nc.sync.dma_start(
                        out=t, in_=w_dram[ct * 128:(ct + 1) * 128, :]
                    )
                    w_sb.append(t)
                for ft in range(4):
                    ps = mm_ps.tile([128, 1024], FP32, tag="mm", name="qps")
                    for ct in range(8):
                        for half in range(2):
                            nc.tensor.matmul(
                                ps[:, half * 512:(half + 1) * 512],
                                w_sb[ct][:, ft * 128:(ft + 1) * 128],
                                act[ct][:, half * 512:(half + 1) * 512],
                                start=(ct == 0),
                                stop=(ct == 7),
                            )
                    nc.vector.tensor_scalar_add(
                        out=dst[ft][:, :],
                        in0=ps[:, :],
                        scalar1=bias_sb[:, ft:ft + 1],
                    )

            # ---- prefetch out-projection weights during attention ----
            wo_sb = []
            for ft in range(4):
                two = big.tile([128, S], BF16, tag="big", name=f"wo{ft}")
                nc.sync.dma_start(out=two, in_=wo[ft * 128:(ft + 1) * 128, :])
                wo_sb.append(two)

            # ---- attention (head pairs row-pack; both q-tiles share one
            #      psum tile so exp runs once per (kt, head)) ----
            oT = [big.tile([128, S], BF16, tag="big", name=f"oT{i}") for i in range(4)]

            def emit_finalize(t, qt, o_sb, recips):
                for hh in range(2):
                        i = hh
                        rd = recips[i]
                        bc_sb = spool.tile([DH, 512], FP32, tag="bc", name="bc_sb")
                        rd_bcast = bass.AP(
                            tensor=rd.tensor, offset=rd.offset,
                            ap=[[0, DH]] + [list(a) for a in rd.ap[1:]],
                        )
                        nc.gpsimd.dma_start(out=bc_sb[:, :], in_=rd_bcast)
                        nc.vector.tensor_mul(
                            out=oT[t][hh * 64:hh * 64 + DH, qt * 512:(qt + 1) * 512],
                            in0=o_sb[i][0:DH, :],
                            in1=bc_sb[:, :],
                        )

            pending = None
            for t in range(4):  # heads 2t, 2t+1
                for qt in range(2):
                    o_ps = [ot_ps.tile([128, 512], FP32, tag="ot", name=f"ops{i}")
                            for i in range(2)]  # per head of the pair
                    prev = None
                    for kt in range(16):
                        KT = KxT[t] if kt < 8 else KyT[t]
                        ks = (kt % 8) * 128
                        sc = mm_ps.tile([128, 1024], FP32, tag="mm", name="sc")
                        for hh in range(2):
                            nc.tensor.matmul(
                                sc[:, hh * 512:(hh + 1) * 512],
                                KT[hh * 64:(hh + 1) * 64, ks:ks + 128],
                                QT[t][hh * 64:(hh + 1) * 64, qt * 512:(qt + 1) * 512],
                                start=True,
                                stop=True,
                            )
                        p2 = ppool.tile([128, 1024], BF16, tag="p", name="p")
                        nc.scalar.activation(out=p2[:, :], in_=sc[:, :], func=EXP)
                        if prev is not None:
                            for hh in range(2):
                                nc.tensor.matmul(
                                    o_ps[hh][0:DH + 1, :],
                                    V[kt - 1][:, 2 * t + hh, :],
                                    prev[:, hh * 512:(hh + 1) * 512],
                                    start=(kt == 1),
                                    stop=False,
                                )
                        prev = p2
                    for hh in range(2):
                        nc.tensor.matmul(
                            o_ps[hh][0:DH + 1, :],
                            V[15][:, 2 * t + hh, :],
                            prev[:, hh * 512:(hh + 1) * 512],
                            start=False,
                            stop=True,
                        )
                    if pending is not None:
                        emit_finalize(*pending)
                    o_sb = []
                    recips = []
                    for i in range(2):
                        ob = spool.tile([DH + 1, 512], FP32, tag="osb", name="osb")
                        nc.vector.tensor_copy(out=ob[:, :], in_=o_ps[i][0:DH + 1, :])
                        o_sb.append(ob)
                    for i in range(2):
                        rf = spool.tile([1, 512], FP32, tag="recipf", name="rf")
                        nc.vector.reciprocal(out=rf[:, :], in_=o_sb[i][DH:DH + 1, :])
                        rd = dpool.tile([1, 512], FP32, name="rd")
                        nc.gpsimd.dma_start(out=rd[:, :], in_=rf[:, :])
                        recips.append(rd)
                    pending = (t, qt, o_sb, recips)
            emit_finalize(*pending)

            # ---- out-projection (transposed domain [m, s]) ----
            for mt in range(8):
                ps = mm_ps.tile([128, 1024], FP32, tag="mm", name="obs")
                for ft in range(4):
                    for half in range(2):
                        nc.tensor.matmul(
                            ps[:, half * 512:(half + 1) * 512],
                            wo_sb[ft][:, mt * 128:(mt + 1) * 128],
                            oT[ft][:, half * 512:(half + 1) * 512],
                            start=(ft == 0),
                            stop=(ft == 3),
                        )
                osb = opool.tile([128, 1024], FP32, tag="osb2", name="osb2")
                nc.vector.tensor_copy(out=osb[:, :], in_=ps[:, :])
                nc.sync.dma_start(
                    out=outT[mt * 128:(mt + 1) * 128, :],
                    in_=osb[:, :],
                )

    _spill_excess_waits(nc)
    return nc


_NC = None


def _get_program():
    global _NC
    if _NC is None:
        _NC = _build_program()
    return _NC


# ---------------------------------------------------------------------------
# host wrapper
# ---------------------------------------------------------------------------

def _prep_in_maps(x, y, W_Kx, b_Kx, W_Qx, b_Qx, W_Vx, b_Vx, W_Ky, b_Ky,
                  W_Vy, b_Vy, W_out, b_out):
    f32 = np.float32
    bf16 = ml_dtypes.bfloat16
    in_maps = []
    for c in range(NCORES):
        b = c // 2
        g = c % 2
        gs = slice(FS * g, FS * (g + 1))
        m = {
            "xT": np.ascontiguousarray(np.asarray(x[b], f32).T).astype(bf16),
            "yT": np.ascontiguousarray(np.asarray(y[b], f32).T).astype(bf16),
            "wq": np.ascontiguousarray((np.asarray(W_Qx, f32)[gs, :] / 8.0).T).astype(bf16),
            "wkx": np.ascontiguousarray(np.asarray(W_Kx, f32)[gs, :].T).astype(bf16),
            "wky": np.ascontiguousarray(np.asarray(W_Ky, f32)[gs, :].T).astype(bf16),
            "wvx": np.ascontiguousarray(np.asarray(W_Vx, f32)[gs, :].T).astype(bf16),
            "wvy": np.ascontiguousarray(np.asarray(W_Vy, f32)[gs, :].T).astype(bf16),
            "wo": np.ascontiguousarray(np.asarray(W_out, f32)[:, gs].T).astype(bf16),
            "bq": np.ascontiguousarray(
                (np.asarray(b_Qx, f32)[gs] / 8.0).reshape(4, 128).T),
            "bkx": np.ascontiguousarray(np.asarray(b_Kx, f32)[gs].reshape(4, 128).T),
            "bky": np.ascontiguousarray(np.asarray(b_Ky, f32)[gs].reshape(4, 128).T),
            "bvx_bc": np.ascontiguousarray(
                np.broadcast_to(np.asarray(b_Vx, f32)[gs], (128, FS))),
            "bvy_bc": np.ascontiguousarray(
                np.broadcast_to(np.asarray(b_Vy, f32)[gs], (128, FS))),
        }
        in_maps.append(m)
    return in_maps


def _assemble(results, b_out):
    B = 4
    out = np.empty((B, S, DIM), np.float32)
    bo = np.asarray(b_out, np.float32)
    for b in range(B):
        acc = results[2 * b]["outT"] + results[2 * b + 1]["outT"]
        out[b] = acc.T + bo
    return out


def kernel(**inputs):
    nc = _get_program()
    in_maps = _prep_in_maps(**inputs)
    res = run_bass_kernel_spmd(nc, in_maps, core_ids=list(range(NCORES)))
    return _assemble(res.results, inputs["b_out"])


def kernel_traced(trace_cores=None, **inputs):
    """Same as kernel() but returns (out, BassKernelResults) with NTFF trace."""
    _register_ntff_hook()
    nc = _get_program()
    in_maps = _prep_in_maps(**inputs)
    res = run_bass_kernel_spmd(
        nc, in_maps, core_ids=list(range(NCORES)), trace=True,
        trace_cores=trace_cores or [0],
    )
    return _assemble(res.results, inputs["b_out"]), res


# revision 19
# speedup vs baseline: 1.4529x; 1.0347x over previous
"""MultiHeadCrossAttention kernel for 8 Trainium2 NeuronCores.

Problem (hardcoded): B=4, Sx=Sy=1024, DIM=1024, H=16, Dh=64, fp32.
  Q = x@W_Qx.T+b_Qx ; K = cat(x@W_Kx.T+b_Kx, y@W_Ky.T+b_Ky) per head
  V = cat(x@W_Vx.T+b_Vx, y@W_Vy.T+b_Vy) ; out = softmax(QK^T/8)V @ W_out.T + b_out

Sharding: core c -> (batch b = c//2, head-group g = c%2 of 8 heads).
Each core computes its batch's attention for its 8 heads plus the partial
out-projection over its 512 features; host sums the two partials per batch
and adds b_out (the "all-reduce after to_out", done in the gather).

Device layout choices (all matmuls natural, zero on-device transposes):
 - activations pre-transposed on host: xT/yT [dim, seq]
 - Q/K projections in transposed domain [feat, seq]  (bias = per-partition)
 - V in natural domain [seq, feat] with host-broadcast bias, plus a ones
   column per head -> AV matmul row 64 yields the softmax denominator
 - scoresT [k, q] via lhsT=KT (d=64 contraction; head pairs row-pack the PE)
 - exp on ACT only (no max subtraction: |scores| <~ 3), normalize via
   PE-broadcast reciprocal, out-projection in transposed domain [m, s]
 - float32r everywhere on the PE: full rate at N=512, ~5e-5 rel err
"""

import os
import sys

os.environ.setdefault("MYCRO_LOCAL_CACHE", "1")
if "/opt/trn_rl_repo" not in sys.path:
    sys.path.insert(0, "/opt/trn_rl_repo")

import ml_dtypes
import numpy as np

import concourse.bass as bass
import concourse.mybir as mybir
import concourse.tile as tile
from concourse import bass_utils
from concourse.bass_utils import run_bass_kernel_spmd

FP32 = mybir.dt.float32
FP32R = mybir.dt.float32r
BF16 = mybir.dt.bfloat16

DIM = 1024
H = 16          # total heads
HG = 8          # heads per core (head-group)
DH = 64
S = 1024        # Sx = Sy
FS = 512        # feature slice per core (HG * DH)
NCORES = 8

# ---------------------------------------------------------------------------
# harness patches (this snapshot's Tile emits >1 wait per instruction in a
# few places; HW instructions hold one wait)
# ---------------------------------------------------------------------------

def _patched_drain_and_barrier(self, tick_clock, wait_clock):
    from bass_rust import ScopedClock

    nc = self.nc
    drain_inst = nc.sync.drain()
    wait_clock.add_sem_waits(
        drain_inst.ins, ScopedClock({None: tick_clock.global_clock})
    )
    si = drain_inst.ins.sync_info
    waits = list(si.on_wait)
    if len(waits) > 1:
        del si.on_wait[1:]
        for w in waits[1:]:
            nop = nc.sync.nop(nofuse=True, hint="drain_wait_spill")
            if nop.ins.sync_info is None:
                nop.ins.sync_info = mybir.SyncInfo(on_wait=[], on_update=[])
            nop.ins.sync_info.on_wait.append(w)

    nc.all_engine_barrier()
    assert self.sems is not None
    popped = nc._tile_sem_poison_stack.pop()
    assert popped is self._sem_poison
    nc.clear_and_free_semaphores(list(self.sems.allocated().values()))
    nc.all_engine_barrier()


def _spill_excess_waits(nc):
    n = 0
    for fn in nc.m.functions:
        for bb in fn.blocks:
            new_insts = []
            for inst in bb.instructions:
                si = getattr(inst, "sync_info", None)
                cap = 2 if isinstance(inst, mybir.InstEventSemaphore) else 1
                if si is not None and si.on_wait and len(si.on_wait) > cap:
                    extras = list(si.on_wait[cap:])
                    del si.on_wait[cap:]
                    for w in extras:
                        new_insts.append(
                            mybir.InstNoOp(
                                name=f"wspill-{nc.next_id()}",
                                engine=inst.engine,
                                ins=[],
                                outs=[],
                                sync_info=mybir.SyncInfo(on_wait=[w], on_update=[]),
                            )
                        )
                        n += 1
                new_insts.append(inst)
            bb.instructions[:] = new_insts
    return n


tile.TileContext._drain_and_barrier = _patched_drain_and_barrier

if os.environ.get("ENABLE_LDW_OPT") == "1":
    _orig_run_command = bass_utils.run_command

    def _run_command_ldw(argv, **kwargs):
        if isinstance(argv, list):
            argv = ["--enable-ldw-opt=true" if a == "--enable-ldw-opt=false" else a
                    for a in argv]
        return _orig_run_command(argv, **kwargs)

    bass_utils.run_command = _run_command_ldw
bass_utils.upload_artifacts = lambda tmpdir: tmpdir  # no S3 in container


def _register_ntff_hook():
    """Best-effort: enables trace=True runs (used by test harness only)."""
    try:
        from antenv.axon_hooks import set_axon_ntff_profile_hook
        sys.path.insert(0, "/root/.axon_site")
        from trn_agent_boot.trn_boot import _ntff_profile_via_ctypes

        set_axon_ntff_profile_hook(
            _ntff_profile_via_ctypes("/opt/axon/libaxon_pjrt.so")
        )
    except Exception:
        pass


# ---------------------------------------------------------------------------
# device program (identical on all 8 cores; per-core data differs)
# ---------------------------------------------------------------------------

def _build_program():
    nc = bass.Bass()

    xT = nc.declare_dram_parameter("xT", [DIM, S], BF16, isOutput=False)
    yT = nc.declare_dram_parameter("yT", [DIM, S], BF16, isOutput=False)
    wq = nc.declare_dram_parameter("wq", [DIM, FS], BF16, isOutput=False)
    wkx = nc.declare_dram_parameter("wkx", [DIM, FS], BF16, isOutput=False)
    wky = nc.declare_dram_parameter("wky", [DIM, FS], BF16, isOutput=False)
    wvx = nc.declare_dram_parameter("wvx", [DIM, FS], BF16, isOutput=False)
    wvy = nc.declare_dram_parameter("wvy", [DIM, FS], BF16, isOutput=False)
    wo = nc.declare_dram_parameter("wo", [FS, DIM], BF16, isOutput=False)
    bq = nc.declare_dram_parameter("bq", [128, 4], FP32, isOutput=False)
    bkx = nc.declare_dram_parameter("bkx", [128, 4], FP32, isOutput=False)
    bky = nc.declare_dram_parameter("bky", [128, 4], FP32, isOutput=False)
    bvx_bc = nc.declare_dram_parameter("bvx_bc", [128, FS], FP32, isOutput=False)
    bvy_bc = nc.declare_dram_parameter("bvy_bc", [128, FS], FP32, isOutput=False)
    outT = nc.declare_dram_parameter("outT", [DIM, S], FP32, isOutput=True)

    EXP = mybir.ActivationFunctionType.Exp

    with tile.TileContext(nc) as tc:
        import contextlib

        with contextlib.ExitStack() as ctx:
            big = ctx.enter_context(tc.tile_pool(name="big", bufs=16))
            wpool = ctx.enter_context(tc.tile_pool(name="wpool", bufs=8))
            qkv = ctx.enter_context(tc.tile_pool(name="qkv", bufs=12))
            vpool = ctx.enter_context(tc.tile_pool(name="vpool", bufs=16))
            ppool = ctx.enter_context(tc.tile_pool(name="ppool", bufs=5))
            opool = ctx.enter_context(tc.tile_pool(name="opool", bufs=2))
            spool = ctx.enter_context(tc.tile_pool(name="spool", bufs=4))
            cpool = ctx.enter_context(tc.tile_pool(name="cpool", bufs=1))
            dpool = ctx.enter_context(tc.tile_pool(name="dpool", bufs=8, space="DRAM"))
            mm_ps = ctx.enter_context(tc.tile_pool(name="mm_ps", bufs=2, space="PSUM"))
            ot_ps = ctx.enter_context(tc.tile_pool(name="ot_ps", bufs=2, space="PSUM"))

            # ---- constants ----
            ones_f32 = cpool.tile([128, 64], FP32, tag="ones_f32")
            nc.vector.memset(ones_f32[:, :], 1.0)
            bq_sb = cpool.tile([128, 4], FP32, tag="bq")
            bkx_sb = cpool.tile([128, 4], FP32, tag="bkx")
            bky_sb = cpool.tile([128, 4], FP32, tag="bky")
            bvx_sb = cpool.tile([128, FS], FP32, tag="bvx")
            bvy_sb = cpool.tile([128, FS], FP32, tag="bvy")
            nc.sync.dma_start(out=bq_sb, in_=bq[:, :])
            nc.sync.dma_start(out=bkx_sb, in_=bkx[:, :])
            nc.sync.dma_start(out=bky_sb, in_=bky[:, :])
            nc.sync.dma_start(out=bvx_sb, in_=bvx_bc[:, :])
            nc.sync.dma_start(out=bvy_sb, in_=bvy_bc[:, :])

            # ---- load activations ----
            xt = []
            wvx_sb = []
            for i in range(8):
                t = big.tile([128, S], BF16, tag="big", name=f"xt{i}")
                nc.sync.dma_start(out=t, in_=xT[i * 128:(i + 1) * 128, :])
                xt.append(t)
                tw = wpool.tile([128, FS], BF16, tag="w", name=f"wvx{i}")
                nc.sync.dma_start(out=tw, in_=wvx[i * 128:(i + 1) * 128, :])
                wvx_sb.append(tw)
            yt = []

            # ---- V projections (natural domain, bias + ones column) ----
            V = [vpool.tile([128, HG, DH + 1], BF16, tag="v", name=f"V{i}") for i in range(16)]
            for src_is_y in (False, True):
                bias_sb = bvy_sb if src_is_y else bvx_sb
                base = 8 if src_is_y else 0
                if src_is_y:
                    w_sb = []
                    for i in range(8):
                        ty = big.tile([128, S], BF16, tag="big", name=f"yt{i}")
                        nc.sync.dma_start(out=ty, in_=yT[i * 128:(i + 1) * 128, :])
                        yt.append(ty)
                        tw = wpool.tile([128, FS], BF16, tag="w", name=f"wvy{i}")
                        nc.sync.dma_start(out=tw, in_=wvy[i * 128:(i + 1) * 128, :])
                        w_sb.append(tw)
                else:
                    w_sb = wvx_sb
                act = yt if src_is_y else xt
                for sg in range(4):  # two s-tiles per psum group
                    ps = mm_ps.tile([128, 1024], FP32, tag="mm", name="vps")
                    for ct in range(8):
                        for half in range(2):
                            st = 2 * sg + half
                            nc.tensor.matmul(
                                ps[:, half * 512:(half + 1) * 512],
                                xt[ct][:, st * 128:(st + 1) * 128] if not src_is_y
                                else yt[ct][:, st * 128:(st + 1) * 128],
                                w_sb[ct][:, :],
                                start=(ct == 0),
                                stop=(ct == 7),
                            )
                    for half in range(2):
                        st = 2 * sg + half
                        vt = V[base + st]
                        nc.vector.tensor_add(
                            out=vt[:, :, 0:DH],
                            in0=ps[:, half * 512:(half + 1) * 512].rearrange(
                                "p (h d) -> p h d", h=HG),
                            in1=bias_sb[:, :].rearrange("p (h d) -> p h d", h=HG),
                        )
                        nc.vector.tensor_copy(
                            out=vt[:, :, DH:DH + 1],
                            in_=ones_f32[:, 0:HG].rearrange("p (h o) -> p h o", o=1),
                        )

            # ---- Q/K projections (transposed domain [feat, seq]) ----
            QT = [qkv.tile([128, S], BF16, tag="qkv", name=f"QT{i}") for i in range(4)]
            KxT = [qkv.tile([128, S], BF16, tag="qkv", name=f"KxT{i}") for i in range(4)]
            KyT = [qkv.tile([128, S], BF16, tag="qkv", name=f"KyT{i}") for i in range(4)]

            for w_dram, act, bias_sb, dst in (
                (wq, xt, bq_sb, QT),
                (wkx, xt, bkx_sb, KxT),
                (wky, yt, bky_sb, KyT),
            ):
                w_sb = []
                for ct in range(8):
                    t = wpool.tile([128, FS], BF16, tag="w", name=f"wp{ct}")
                    nc.sync.dma_start(
                        out=t, in_=w_dram[ct * 128:(ct + 1) * 128, :]
                    )
                    w_sb.append(t)
                for ft in range(4):
                    ps = mm_ps.tile([128, 1024], FP32, tag="mm", name="qps")
                    for ct in range(8):
                        for half in range(2):
                            nc.tensor.matmul(
                                ps[:, half * 512:(half + 1) * 512],
                                w_sb[ct][:, ft * 128:(ft + 1) * 128],
                                act[ct][:, half * 512:(half + 1) * 512],
                                start=(ct == 0),
                                stop=(ct == 7),
                            )
                    nc.vector.tensor_scalar_add(
                        out=dst[ft][:, :],
                        in0=ps[:, :],
                        scalar1=bias_sb[:, ft:ft + 1],
                    )

            # ---- prefetch out-projection weights during attention ----
            wo_sb = []
            for ft in range(4):
                two = big.tile([128, S], BF16, tag="big", name=f"wo{ft}")
                nc.sync.dma_start(out=two, in_=wo[ft * 128:(ft + 1) * 128, :])
                wo_sb.append(two)

            # ---- attention (head pairs row-pack; both q-tiles share one
            #      psum tile so exp runs once per (kt, head)) ----
            oT = [big.tile([128, S], BF16, tag="big", name=f"oT{i}") for i in range(4)]

            def emit_finalize(t, qt, o_sb, recips):
                for hh in range(2):
                        i = hh
                        rd = recips[i]
                        bc_sb = spool.tile([DH, 512], FP32, tag="bc", name="bc_sb")
                        rd_bcast = bass.AP(
                            tensor=rd.tensor, offset=rd.offset,
                            ap=[[0, DH]] + [list(a) for a in rd.ap[1:]],
                        )
                        nc.gpsimd.dma_start(out=bc_sb[:, :], in_=rd_bcast)
                        nc.vector.tensor_mul(
                            out=oT[t][hh * 64:hh * 64 + DH, qt * 512:(qt + 1) * 512],
                            in0=o_sb[i][0:DH, :],
                            in1=bc_sb[:, :],
                        )

            pending = None
            for t in range(4):  # heads 2t, 2t+1
                for qt in range(2):
                    o_ps = [ot_ps.tile([128, 512], FP32, tag="ot", name=f"ops{i}")
                            for i in range(2)]  # per head of the pair
                    prev = None
                    for kt in range(16):
                        KT = KxT[t] if kt < 8 else KyT[t]
                        ks = (kt % 8) * 128
                        sc = mm_ps.tile([128, 1024], FP32, tag="mm", name="sc")
                        for hh in range(2):
                            nc.tensor.matmul(
                                sc[:, hh * 512:(hh + 1) * 512],
                                KT[hh * 64:(hh + 1) * 64, ks:ks + 128],
                                QT[t][hh * 64:(hh + 1) * 64, qt * 512:(qt + 1) * 512],
                                start=True,
                                stop=True,
                            )
                        p2 = ppool.tile([128, 1024], BF16, tag="p", name="p")
                        nc.scalar.activation(out=p2[:, :], in_=sc[:, :], func=EXP)
                        if prev is not None:
                            for hh in range(2):
                                nc.tensor.matmul(
                                    o_ps[hh][0:DH + 1, :],
                                    V[kt - 1][:, 2 * t + hh, :],
                                    prev[:, hh * 512:(hh + 1) * 512],
                                    start=(kt == 1),
                                    stop=False,
                                )
                        prev = p2
                    for hh in range(2):
                        nc.tensor.matmul(
                            o_ps[hh][0:DH + 1, :],
                            V[15][:, 2 * t + hh, :],
                            prev[:, hh * 512:(hh + 1) * 512],
                            start=False,
                            stop=True,
                        )
                    if pending is not None:
                        emit_finalize(*pending)
                    o_sb = []
                    recips = []
                    for i in range(2):
                        ob = spool.tile([DH + 1, 512], FP32, tag="osb", name="osb")
                        nc.vector.tensor_copy(out=ob[:, :], in_=o_ps[i][0:DH + 1, :])
                        o_sb.append(ob)
                    for i in range(2):
                        rf = spool.tile([1, 512], FP32, tag="recipf", name="rf")
                        nc.vector.reciprocal(out=rf[:, :], in_=o_sb[i][DH:DH + 1, :])
                        rd = dpool.tile([1, 512], FP32, name="rd")
                        nc.gpsimd.dma_start(out=rd[:, :], in_=rf[:, :])
                        recips.append(rd)
                    pending = (t, qt, o_sb, recips)
            emit_finalize(*pending)

            # ---- out-projection (transposed domain [m, s]) ----
            for mt in range(8):
                ps = mm_ps.tile([128, 1024], FP32, tag="mm", name="obs")
                for ft in range(4):
                    for half in range(2):
                        nc.tensor.matmul(
                            ps[:, half * 512:(half + 1) * 512],
                            wo_sb[ft][:, mt * 128:(mt + 1) * 128],
                            oT[ft][:, half * 512:(half + 1) * 512],
                            start=(ft == 0),
                            stop=(ft == 3),
                        )
                osb = opool.tile([128, 1024], FP32, tag="osb2", name="osb2")
                nc.vector.tensor_copy(out=osb[:, :], in_=ps[:, :])
                nc.sync.dma_start(
                    out=outT[mt * 128:(mt + 1) * 128, :],
                    in_=osb[:, :],
                )

    _spill_excess_waits(nc)
    return nc


_NC = None


def _get_program():
    global _NC
    if _NC is None:
        _NC = _build_program()
    return _NC


# ---------------------------------------------------------------------------
# host wrapper
# ---------------------------------------------------------------------------

def _prep_in_maps(x, y, W_Kx, b_Kx, W_Qx, b_Qx, W_Vx, b_Vx, W_Ky, b_Ky,
                  W_Vy, b_Vy, W_out, b_out):
    f32 = np.float32
    bf16 = ml_dtypes.bfloat16
    in_maps = []
    for c in range(NCORES):
        b = c // 2
        g = c % 2
        gs = slice(FS * g, FS * (g + 1))
        m = {
            "xT": np.ascontiguousarray(np.asarray(x[b], f32).T).astype(bf16),
            "yT": np.ascontiguousarray(np.asarray(y[b], f32).T).astype(bf16),
            "wq": np.ascontiguousarray((np.asarray(W_Qx, f32)[gs, :] / 8.0).T).astype(bf16),
            "wkx": np.ascontiguousarray(np.asarray(W_Kx, f32)[gs, :].T).astype(bf16),
            "wky": np.ascontiguousarray(np.asarray(W_Ky, f32)[gs, :].T).astype(bf16),
            "wvx": np.ascontiguousarray(np.asarray(W_Vx, f32)[gs, :].T).astype(bf16),
            "wvy": np.ascontiguousarray(np.asarray(W_Vy, f32)[gs, :].T).astype(bf16),
            "wo": np.ascontiguousarray(np.asarray(W_out, f32)[:, gs].T).astype(bf16),
            "bq": np.ascontiguousarray(
                (np.asarray(b_Qx, f32)[gs] / 8.0).reshape(4, 128).T),
            "bkx": np.ascontiguousarray(np.asarray(b_Kx, f32)[gs].reshape(4, 128).T),
            "bky": np.ascontiguousarray(np.asarray(b_Ky, f32)[gs].reshape(4, 128).T),
            "bvx_bc": np.ascontiguousarray(
                np.broadcast_to(np.asarray(b_Vx, f32)[gs], (128, FS))),
            "bvy_bc": np.ascontiguousarray(
                np.broadcast_to(np.asarray(b_Vy, f32)[gs], (128, FS))),
        }
        in_maps.append(m)
    return in_maps


def _assemble(results, b_out):
    B = 4
    out = np.empty((B, S, DIM), np.float32)
    bo = np.asarray(b_out, np.float32)
    for b in range(B):
        acc = results[2 * b]["outT"] + results[2 * b + 1]["outT"]
        out[b] = acc.T + bo
    return out


def kernel(**inputs):
    nc = _get_program()
    in_maps = _prep_in_maps(**inputs)
    res = run_bass_kernel_spmd(nc, in_maps, core_ids=list(range(NCORES)))
    return _assemble(res.results, inputs["b_out"])


def kernel_traced(trace_cores=None, **inputs):
    """Same as kernel() but returns (out, BassKernelResults) with NTFF trace."""
    _register_ntff_hook()
    nc = _get_program()
    in_maps = _prep_in_maps(**inputs)
    res = run_bass_kernel_spmd(
        nc, in_maps, core_ids=list(range(NCORES)), trace=True,
        trace_cores=trace_cores or [0],
    )
    return _assemble(res.results, inputs["b_out"]), res


# revision 20
# speedup vs baseline: 1.6063x; 1.1056x over previous
"""MultiHeadCrossAttention kernel for 8 Trainium2 NeuronCores.

Problem (hardcoded): B=4, Sx=Sy=1024, DIM=1024, H=16, Dh=64, fp32.
  Q = x@W_Qx.T+b_Qx ; K = cat(x@W_Kx.T+b_Kx, y@W_Ky.T+b_Ky) per head
  V = cat(x@W_Vx.T+b_Vx, y@W_Vy.T+b_Vy) ; out = softmax(QK^T/8)V @ W_out.T + b_out

Sharding: core c -> (batch b = c//2, head-group g = c%2 of 8 heads).
Each core computes its batch's attention for its 8 heads plus the partial
out-projection over its 512 features; host sums the two partials per batch
and adds b_out (the "all-reduce after to_out", done in the gather).

Device layout choices (all matmuls natural, zero on-device transposes):
 - activations pre-transposed on host: xT/yT [dim, seq]
 - Q/K projections in transposed domain [feat, seq]  (bias = per-partition)
 - V in natural domain [seq, feat] with host-broadcast bias, plus a ones
   column per head -> AV matmul row 64 yields the softmax denominator
 - scoresT [k, q] via lhsT=KT (d=64 contraction; head pairs row-pack the PE)
 - exp on ACT only (no max subtraction: |scores| <~ 3), normalize via
   PE-broadcast reciprocal, out-projection in transposed domain [m, s]
 - float32r everywhere on the PE: full rate at N=512, ~5e-5 rel err
"""

import os
import sys

os.environ.setdefault("MYCRO_LOCAL_CACHE", "1")
if "/opt/trn_rl_repo" not in sys.path:
    sys.path.insert(0, "/opt/trn_rl_repo")

import ml_dtypes
import numpy as np

import concourse.bass as bass
import concourse.mybir as mybir
import concourse.tile as tile
from concourse import bass_utils
from concourse.bass_utils import run_bass_kernel_spmd

FP32 = mybir.dt.float32
FP32R = mybir.dt.float32r
BF16 = mybir.dt.bfloat16

DIM = 1024
H = 16          # total heads
HG = 8          # heads per core (head-group)
DH = 64
S = 1024        # Sx = Sy
FS = 512        # feature slice per core (HG * DH)
NCORES = 8

# ---------------------------------------------------------------------------
# harness patches (this snapshot's Tile emits >1 wait per instruction in a
# few places; HW instructions hold one wait)
# ---------------------------------------------------------------------------

def _patched_drain_and_barrier(self, tick_clock, wait_clock):
    from bass_rust import ScopedClock

    nc = self.nc
    drain_inst = nc.sync.drain()
    wait_clock.add_sem_waits(
        drain_inst.ins, ScopedClock({None: tick_clock.global_clock})
    )
    si = drain_inst.ins.sync_info
    waits = list(si.on_wait)
    if len(waits) > 1:
        del si.on_wait[1:]
        for w in waits[1:]:
            nop = nc.sync.nop(nofuse=True, hint="drain_wait_spill")
            if nop.ins.sync_info is None:
                nop.ins.sync_info = mybir.SyncInfo(on_wait=[], on_update=[])
            nop.ins.sync_info.on_wait.append(w)

    nc.all_engine_barrier()
    assert self.sems is not None
    popped = nc._tile_sem_poison_stack.pop()
    assert popped is self._sem_poison
    nc.clear_and_free_semaphores(list(self.sems.allocated().values()))
    nc.all_engine_barrier()


def _spill_excess_waits(nc):
    n = 0
    for fn in nc.m.functions:
        for bb in fn.blocks:
            new_insts = []
            for inst in bb.instructions:
                si = getattr(inst, "sync_info", None)
                cap = 2 if isinstance(inst, mybir.InstEventSemaphore) else 1
                if si is not None and si.on_wait and len(si.on_wait) > cap:
                    extras = list(si.on_wait[cap:])
                    del si.on_wait[cap:]
                    for w in extras:
                        new_insts.append(
                            mybir.InstNoOp(
                                name=f"wspill-{nc.next_id()}",
                                engine=inst.engine,
                                ins=[],
                                outs=[],
                                sync_info=mybir.SyncInfo(on_wait=[w], on_update=[]),
                            )
                        )
                        n += 1
                new_insts.append(inst)
            bb.instructions[:] = new_insts
    return n


tile.TileContext._drain_and_barrier = _patched_drain_and_barrier

if os.environ.get("ENABLE_LDW_OPT") == "1":
    _orig_run_command = bass_utils.run_command

    def _run_command_ldw(argv, **kwargs):
        if isinstance(argv, list):
            argv = ["--enable-ldw-opt=true" if a == "--enable-ldw-opt=false" else a
                    for a in argv]
        return _orig_run_command(argv, **kwargs)

    bass_utils.run_command = _run_command_ldw
bass_utils.upload_artifacts = lambda tmpdir: tmpdir  # no S3 in container


def _register_ntff_hook():
    """Best-effort: enables trace=True runs (used by test harness only)."""
    try:
        from antenv.axon_hooks import set_axon_ntff_profile_hook
        sys.path.insert(0, "/root/.axon_site")
        from trn_agent_boot.trn_boot import _ntff_profile_via_ctypes

        set_axon_ntff_profile_hook(
            _ntff_profile_via_ctypes("/opt/axon/libaxon_pjrt.so")
        )
    except Exception:
        pass


# ---------------------------------------------------------------------------
# device program (identical on all 8 cores; per-core data differs)
# ---------------------------------------------------------------------------

def _build_program():
    nc = bass.Bass()

    xT = nc.declare_dram_parameter("xT", [DIM, S], BF16, isOutput=False)
    yT = nc.declare_dram_parameter("yT", [DIM, S], BF16, isOutput=False)
    wq = nc.declare_dram_parameter("wq", [DIM, FS], BF16, isOutput=False)
    wkx = nc.declare_dram_parameter("wkx", [DIM, FS], BF16, isOutput=False)
    wky = nc.declare_dram_parameter("wky", [DIM, FS], BF16, isOutput=False)
    wvx = nc.declare_dram_parameter("wvx", [DIM, FS], BF16, isOutput=False)
    wvy = nc.declare_dram_parameter("wvy", [DIM, FS], BF16, isOutput=False)
    wo = nc.declare_dram_parameter("wo", [FS, DIM], BF16, isOutput=False)
    bq = nc.declare_dram_parameter("bq", [128, 4], FP32, isOutput=False)
    bkx = nc.declare_dram_parameter("bkx", [128, 4], FP32, isOutput=False)
    bky = nc.declare_dram_parameter("bky", [128, 4], FP32, isOutput=False)
    bvx_bc = nc.declare_dram_parameter("bvx_bc", [128, FS], FP32, isOutput=False)
    bvy_bc = nc.declare_dram_parameter("bvy_bc", [128, FS], FP32, isOutput=False)
    outT = nc.declare_dram_parameter("outT", [DIM, S], FP32, isOutput=True)

    EXP = mybir.ActivationFunctionType.Exp

    with tile.TileContext(nc) as tc:
        import contextlib

        with contextlib.ExitStack() as ctx:
            big = ctx.enter_context(tc.tile_pool(name="big", bufs=24))
            wpool = ctx.enter_context(tc.tile_pool(name="wpool", bufs=26))
            qkv = ctx.enter_context(tc.tile_pool(name="qkv", bufs=12))
            vpool = ctx.enter_context(tc.tile_pool(name="vpool", bufs=16))
            ppool = ctx.enter_context(tc.tile_pool(name="ppool", bufs=5))
            opool = ctx.enter_context(tc.tile_pool(name="opool", bufs=2))
            spool = ctx.enter_context(tc.tile_pool(name="spool", bufs=4))
            cpool = ctx.enter_context(tc.tile_pool(name="cpool", bufs=1))
            dpool = ctx.enter_context(tc.tile_pool(name="dpool", bufs=8, space="DRAM"))
            mm_ps = ctx.enter_context(tc.tile_pool(name="mm_ps", bufs=3, space="PSUM"))
            ot_ps = ctx.enter_context(tc.tile_pool(name="ot_ps", bufs=2, space="PSUM"))

            # ---- constants ----
            ones_f32 = cpool.tile([128, 64], FP32, tag="ones_f32")
            nc.vector.memset(ones_f32[:, :], 1.0)
            bq_sb = cpool.tile([128, 4], FP32, tag="bq")
            bkx_sb = cpool.tile([128, 4], FP32, tag="bkx")
            bky_sb = cpool.tile([128, 4], FP32, tag="bky")
            bvx_sb = cpool.tile([128, FS], FP32, tag="bvx")
            bvy_sb = cpool.tile([128, FS], FP32, tag="bvy")
            nc.sync.dma_start(out=bq_sb, in_=bq[:, :])
            nc.sync.dma_start(out=bkx_sb, in_=bkx[:, :])
            nc.sync.dma_start(out=bky_sb, in_=bky[:, :])
            nc.sync.dma_start(out=bvx_sb, in_=bvx_bc[:, :])
            nc.sync.dma_start(out=bvy_sb, in_=bvy_bc[:, :])

            # ---- load activations ----
            xt = []
            wvx_sb = []
            for i in range(8):
                t = big.tile([128, S], BF16, tag="big", name=f"xt{i}")
                nc.sync.dma_start(out=t, in_=xT[i * 128:(i + 1) * 128, :])
                xt.append(t)
                tw = wpool.tile([128, FS], BF16, tag="w", name=f"wvx{i}")
                nc.sync.dma_start(out=tw, in_=wvx[i * 128:(i + 1) * 128, :])
                wvx_sb.append(tw)
            yt = []

            # ---- V projections (natural domain, bias + ones column) ----
            V = [vpool.tile([128, HG, DH + 1], BF16, tag="v", name=f"V{i}") for i in range(16)]
            for src_is_y in (False, True):
                bias_sb = bvy_sb if src_is_y else bvx_sb
                base = 8 if src_is_y else 0
                if src_is_y:
                    w_sb = []
                    for i in range(8):
                        ty = big.tile([128, S], BF16, tag="big", name=f"yt{i}")
                        nc.sync.dma_start(out=ty, in_=yT[i * 128:(i + 1) * 128, :])
                        yt.append(ty)
                        tw = wpool.tile([128, FS], BF16, tag="w", name=f"wvy{i}")
                        nc.sync.dma_start(out=tw, in_=wvy[i * 128:(i + 1) * 128, :])
                        w_sb.append(tw)
                else:
                    w_sb = wvx_sb
                act = yt if src_is_y else xt
                for sg in range(4):  # two s-tiles per psum group
                    ps = mm_ps.tile([128, 1024], FP32, tag="mm", name="vps")
                    for ct in range(8):
                        for half in range(2):
                            st = 2 * sg + half
                            nc.tensor.matmul(
                                ps[:, half * 512:(half + 1) * 512],
                                xt[ct][:, st * 128:(st + 1) * 128] if not src_is_y
                                else yt[ct][:, st * 128:(st + 1) * 128],
                                w_sb[ct][:, :],
                                start=(ct == 0),
                                stop=(ct == 7),
                            )
                    for half in range(2):
                        st = 2 * sg + half
                        vt = V[base + st]
                        nc.vector.tensor_add(
                            out=vt[:, :, 0:DH],
                            in0=ps[:, half * 512:(half + 1) * 512].rearrange(
                                "p (h d) -> p h d", h=HG),
                            in1=bias_sb[:, :].rearrange("p (h d) -> p h d", h=HG),
                        )
                        nc.vector.tensor_copy(
                            out=vt[:, :, DH:DH + 1],
                            in_=ones_f32[:, 0:HG].rearrange("p (h o) -> p h o", o=1),
                        )

            # ---- Q/K projections (transposed domain [feat, seq]) ----
            QT = [qkv.tile([128, S], BF16, tag="qkv", name=f"QT{i}") for i in range(4)]
            KxT = [qkv.tile([128, S], BF16, tag="qkv", name=f"KxT{i}") for i in range(4)]
            KyT = [qkv.tile([128, S], BF16, tag="qkv", name=f"KyT{i}") for i in range(4)]

            qk_w = []
            for pi, w_dram in enumerate((wq, wkx, wky)):
                lst = []
                for ct in range(8):
                    tw = wpool.tile([128, FS], BF16, tag="w", name=f"wp{pi}_{ct}")
                    nc.sync.dma_start(
                        out=tw, in_=w_dram[ct * 128:(ct + 1) * 128, :]
                    )
                    lst.append(tw)
                qk_w.append(lst)
            qk_act = [xt, xt, yt]
            qk_bias = [bq_sb, bkx_sb, bky_sb]
            qk_dst = [QT, KxT, KyT]
            qk_ps = {}

            def emit_qk_half(pi, ft, half):
                key = (pi, ft)
                if key not in qk_ps:
                    qk_ps[key] = mm_ps.tile(
                        [128, 1024], FP32, tag="mm", name=f"qkps{pi}_{ft}"
                    )
                ps = qk_ps[key]
                w_sb = qk_w[pi]
                act = qk_act[pi]
                for ct in (range(0, 4) if half == 0 else range(4, 8)):
                    for h2 in range(2):
                        nc.tensor.matmul(
                            ps[:, h2 * 512:(h2 + 1) * 512],
                            w_sb[ct][:, ft * 128:(ft + 1) * 128],
                            act[ct][:, h2 * 512:(h2 + 1) * 512],
                            start=(ct == 0),
                            stop=(ct == 7),
                        )
                if half == 1:
                    nc.vector.tensor_scalar_add(
                        out=qk_dst[pi][ft][:, :],
                        in0=ps[:, :],
                        scalar1=qk_bias[pi][:, ft:ft + 1],
                    )
                    del qk_ps[key]

            # upfront: ft=0 for all projections, plus all fts of proj 2 (wky)
            for pi in range(3):
                emit_qk_half(pi, 0, 0)
                emit_qk_half(pi, 0, 1)
            for ft in range(1, 4):
                emit_qk_half(2, ft, 0)
                emit_qk_half(2, ft, 1)

            # remaining 6 groups ride the attention phase's spare PE cycles
            qk_fillers = {(t, qt): (qt, t + 1) for t in range(3) for qt in range(2)}

            # ---- prefetch out-projection weights during attention ----
            wo_sb = []
            for ft in range(4):
                two = big.tile([128, S], BF16, tag="big", name=f"wo{ft}")
                nc.sync.dma_start(out=two, in_=wo[ft * 128:(ft + 1) * 128, :])
                wo_sb.append(two)

            # ---- attention (head pairs row-pack; both q-tiles share one
            #      psum tile so exp runs once per (kt, head)) ----
            oT = [big.tile([128, S], BF16, tag="big", name=f"oT{i}") for i in range(4)]

            def emit_finalize(t, qt, o_sb, recips):
                for hh in range(2):
                        i = hh
                        rd = recips[i]
                        bc_sb = spool.tile([DH, 512], FP32, tag="bc", name="bc_sb")
                        rd_bcast = bass.AP(
                            tensor=rd.tensor, offset=rd.offset,
                            ap=[[0, DH]] + [list(a) for a in rd.ap[1:]],
                        )
                        nc.gpsimd.dma_start(out=bc_sb[:, :], in_=rd_bcast)
                        nc.vector.tensor_mul(
                            out=oT[t][hh * 64:hh * 64 + DH, qt * 512:(qt + 1) * 512],
                            in0=o_sb[i][0:DH, :],
                            in1=bc_sb[:, :],
                        )

            pending = None
            for t in range(4):  # heads 2t, 2t+1
                for qt in range(2):
                    o_ps = [ot_ps.tile([128, 512], FP32, tag="ot", name=f"ops{i}")
                            for i in range(2)]  # per head of the pair
                    prev = None
                    for kt in range(16):
                        KT = KxT[t] if kt < 8 else KyT[t]
                        ks = (kt % 8) * 128
                        sc = mm_ps.tile([128, 1024], FP32, tag="mm", name="sc")
                        for hh in range(2):
                            nc.tensor.matmul(
                                sc[:, hh * 512:(hh + 1) * 512],
                                KT[hh * 64:(hh + 1) * 64, ks:ks + 128],
                                QT[t][hh * 64:(hh + 1) * 64, qt * 512:(qt + 1) * 512],
                                start=True,
                                stop=True,
                            )
                        p2 = ppool.tile([128, 1024], BF16, tag="p", name="p")
                        nc.scalar.activation(out=p2[:, :], in_=sc[:, :], func=EXP)
                        if (t, qt) in qk_fillers and kt in (4, 11):
                            fpi, fft = qk_fillers[(t, qt)]
                            emit_qk_half(fpi, fft, 0 if kt == 4 else 1)
                        if prev is not None:
                            for hh in range(2):
                                nc.tensor.matmul(
                                    o_ps[hh][0:DH + 1, :],
                                    V[kt - 1][:, 2 * t + hh, :],
                                    prev[:, hh * 512:(hh + 1) * 512],
                                    start=(kt == 1),
                                    stop=False,
                                )
                        prev = p2
                    for hh in range(2):
                        nc.tensor.matmul(
                            o_ps[hh][0:DH + 1, :],
                            V[15][:, 2 * t + hh, :],
                            prev[:, hh * 512:(hh + 1) * 512],
                            start=False,
                            stop=True,
                        )
                    if pending is not None:
                        emit_finalize(*pending)
                    o_sb = []
                    recips = []
                    for i in range(2):
                        ob = spool.tile([DH + 1, 512], FP32, tag="osb", name="osb")
                        nc.vector.tensor_copy(out=ob[:, :], in_=o_ps[i][0:DH + 1, :])
                        o_sb.append(ob)
                    for i in range(2):
                        rf = spool.tile([1, 512], FP32, tag="recipf", name="rf")
                        nc.vector.reciprocal(out=rf[:, :], in_=o_sb[i][DH:DH + 1, :])
                        rd = dpool.tile([1, 512], FP32, name="rd")
                        nc.gpsimd.dma_start(out=rd[:, :], in_=rf[:, :])
                        recips.append(rd)
                    pending = (t, qt, o_sb, recips)
            emit_finalize(*pending)

            # ---- out-projection (transposed domain [m, s]) ----
            for mt in range(8):
                ps = mm_ps.tile([128, 1024], FP32, tag="mm", name="obs")
                for ft in range(4):
                    for half in range(2):
                        nc.tensor.matmul(
                            ps[:, half * 512:(half + 1) * 512],
                            wo_sb[ft][:, mt * 128:(mt + 1) * 128],
                            oT[ft][:, half * 512:(half + 1) * 512],
                            start=(ft == 0),
                            stop=(ft == 3),
                        )
                osb = opool.tile([128, 1024], FP32, tag="osb2", name="osb2")
                nc.vector.tensor_copy(out=osb[:, :], in_=ps[:, :])
                nc.sync.dma_start(
                    out=outT[mt * 128:(mt + 1) * 128, :],
                    in_=osb[:, :],
                )

    _spill_excess_waits(nc)
    return nc


_NC = None


def _get_program():
    global _NC
    if _NC is None:
        _NC = _build_program()
    return _NC


# ---------------------------------------------------------------------------
# host wrapper
# ---------------------------------------------------------------------------

def _prep_in_maps(x, y, W_Kx, b_Kx, W_Qx, b_Qx, W_Vx, b_Vx, W_Ky, b_Ky,
                  W_Vy, b_Vy, W_out, b_out):
    f32 = np.float32
    bf16 = ml_dtypes.bfloat16
    in_maps = []
    for c in range(NCORES):
        b = c // 2
        g = c % 2
        gs = slice(FS * g, FS * (g + 1))
        m = {
            "xT": np.ascontiguousarray(np.asarray(x[b], f32).T).astype(bf16),
            "yT": np.ascontiguousarray(np.asarray(y[b], f32).T).astype(bf16),
            "wq": np.ascontiguousarray((np.asarray(W_Qx, f32)[gs, :] / 8.0).T).astype(bf16),
            "wkx": np.ascontiguousarray(np.asarray(W_Kx, f32)[gs, :].T).astype(bf16),
            "wky": np.ascontiguousarray(np.asarray(W_Ky, f32)[gs, :].T).astype(bf16),
            "wvx": np.ascontiguousarray(np.asarray(W_Vx, f32)[gs, :].T).astype(bf16),
            "wvy": np.ascontiguousarray(np.asarray(W_Vy, f32)[gs, :].T).astype(bf16),
            "wo": np.ascontiguousarray(np.asarray(W_out, f32)[:, gs].T).astype(bf16),
            "bq": np.ascontiguousarray(
                (np.asarray(b_Qx, f32)[gs] / 8.0).reshape(4, 128).T),
            "bkx": np.ascontiguousarray(np.asarray(b_Kx, f32)[gs].reshape(4, 128).T),
            "bky": np.ascontiguousarray(np.asarray(b_Ky, f32)[gs].reshape(4, 128).T),
            "bvx_bc": np.ascontiguousarray(
                np.broadcast_to(np.asarray(b_Vx, f32)[gs], (128, FS))),
            "bvy_bc": np.ascontiguousarray(
                np.broadcast_to(np.asarray(b_Vy, f32)[gs], (128, FS))),
        }
        in_maps.append(m)
    return in_maps


def _assemble(results, b_out):
    B = 4
    out = np.empty((B, S, DIM), np.float32)
    bo = np.asarray(b_out, np.float32)
    for b in range(B):
        acc = results[2 * b]["outT"] + results[2 * b + 1]["outT"]
        out[b] = acc.T + bo
    return out


def kernel(**inputs):
    nc = _get_program()
    in_maps = _prep_in_maps(**inputs)
    res = run_bass_kernel_spmd(nc, in_maps, core_ids=list(range(NCORES)))
    return _assemble(res.results, inputs["b_out"])


def kernel_traced(trace_cores=None, **inputs):
    """Same as kernel() but returns (out, BassKernelResults) with NTFF trace."""
    _register_ntff_hook()
    nc = _get_program()
    in_maps = _prep_in_maps(**inputs)
    res = run_bass_kernel_spmd(
        nc, in_maps, core_ids=list(range(NCORES)), trace=True,
        trace_cores=trace_cores or [0],
    )
    return _assemble(res.results, inputs["b_out"]), res


# revision 21
# speedup vs baseline: 1.6155x; 1.0058x over previous
"""MultiHeadCrossAttention kernel for 8 Trainium2 NeuronCores.

Problem (hardcoded): B=4, Sx=Sy=1024, DIM=1024, H=16, Dh=64, fp32.
  Q = x@W_Qx.T+b_Qx ; K = cat(x@W_Kx.T+b_Kx, y@W_Ky.T+b_Ky) per head
  V = cat(x@W_Vx.T+b_Vx, y@W_Vy.T+b_Vy) ; out = softmax(QK^T/8)V @ W_out.T + b_out

Sharding: core c -> (batch b = c//2, head-group g = c%2 of 8 heads).
Each core computes its batch's attention for its 8 heads plus the partial
out-projection over its 512 features; host sums the two partials per batch
and adds b_out (the "all-reduce after to_out", done in the gather).

Device layout choices (all matmuls natural, zero on-device transposes):
 - activations pre-transposed on host: xT/yT [dim, seq]
 - Q/K projections in transposed domain [feat, seq]  (bias = per-partition)
 - V in natural domain [seq, feat] with host-broadcast bias, plus a ones
   column per head -> AV matmul row 64 yields the softmax denominator
 - scoresT [k, q] via lhsT=KT (d=64 contraction; head pairs row-pack the PE)
 - exp on ACT only (no max subtraction: |scores| <~ 3), normalize via
   PE-broadcast reciprocal, out-projection in transposed domain [m, s]
 - float32r everywhere on the PE: full rate at N=512, ~5e-5 rel err
"""

import os
import sys

os.environ.setdefault("MYCRO_LOCAL_CACHE", "1")
if "/opt/trn_rl_repo" not in sys.path:
    sys.path.insert(0, "/opt/trn_rl_repo")

import ml_dtypes
import numpy as np

import concourse.bass as bass
import concourse.mybir as mybir
import concourse.tile as tile
from concourse import bass_utils
from concourse.bass_utils import run_bass_kernel_spmd

FP32 = mybir.dt.float32
FP32R = mybir.dt.float32r
BF16 = mybir.dt.bfloat16

DIM = 1024
H = 16          # total heads
HG = 8          # heads per core (head-group)
DH = 64
S = 1024        # Sx = Sy
FS = 512        # feature slice per core (HG * DH)
NCORES = 8

# ---------------------------------------------------------------------------
# harness patches (this snapshot's Tile emits >1 wait per instruction in a
# few places; HW instructions hold one wait)
# ---------------------------------------------------------------------------

def _patched_drain_and_barrier(self, tick_clock, wait_clock):
    from bass_rust import ScopedClock

    nc = self.nc
    drain_inst = nc.sync.drain()
    wait_clock.add_sem_waits(
        drain_inst.ins, ScopedClock({None: tick_clock.global_clock})
    )
    si = drain_inst.ins.sync_info
    waits = list(si.on_wait)
    if len(waits) > 1:
        del si.on_wait[1:]
        for w in waits[1:]:
            nop = nc.sync.nop(nofuse=True, hint="drain_wait_spill")
            if nop.ins.sync_info is None:
                nop.ins.sync_info = mybir.SyncInfo(on_wait=[], on_update=[])
            nop.ins.sync_info.on_wait.append(w)

    nc.all_engine_barrier()
    assert self.sems is not None
    popped = nc._tile_sem_poison_stack.pop()
    assert popped is self._sem_poison
    nc.clear_and_free_semaphores(list(self.sems.allocated().values()))
    nc.all_engine_barrier()


def _spill_excess_waits(nc):
    n = 0
    for fn in nc.m.functions:
        for bb in fn.blocks:
            new_insts = []
            for inst in bb.instructions:
                si = getattr(inst, "sync_info", None)
                cap = 2 if isinstance(inst, mybir.InstEventSemaphore) else 1
                if si is not None and si.on_wait and len(si.on_wait) > cap:
                    extras = list(si.on_wait[cap:])
                    del si.on_wait[cap:]
                    for w in extras:
                        new_insts.append(
                            mybir.InstNoOp(
                                name=f"wspill-{nc.next_id()}",
                                engine=inst.engine,
                                ins=[],
                                outs=[],
                                sync_info=mybir.SyncInfo(on_wait=[w], on_update=[]),
                            )
                        )
                        n += 1
                new_insts.append(inst)
            bb.instructions[:] = new_insts
    return n


tile.TileContext._drain_and_barrier = _patched_drain_and_barrier

if os.environ.get("ENABLE_LDW_OPT") == "1":
    _orig_run_command = bass_utils.run_command

    def _run_command_ldw(argv, **kwargs):
        if isinstance(argv, list):
            argv = ["--enable-ldw-opt=true" if a == "--enable-ldw-opt=false" else a
                    for a in argv]
        return _orig_run_command(argv, **kwargs)

    bass_utils.run_command = _run_command_ldw
bass_utils.upload_artifacts = lambda tmpdir: tmpdir  # no S3 in container


def _register_ntff_hook():
    """Best-effort: enables trace=True runs (used by test harness only)."""
    try:
        from antenv.axon_hooks import set_axon_ntff_profile_hook
        sys.path.insert(0, "/root/.axon_site")
        from trn_agent_boot.trn_boot import _ntff_profile_via_ctypes

        set_axon_ntff_profile_hook(
            _ntff_profile_via_ctypes("/opt/axon/libaxon_pjrt.so")
        )
    except Exception:
        pass


# ---------------------------------------------------------------------------
# device program (identical on all 8 cores; per-core data differs)
# ---------------------------------------------------------------------------

def _build_program():
    nc = bass.Bass()

    xT = nc.declare_dram_parameter("xT", [DIM, S], BF16, isOutput=False)
    yT = nc.declare_dram_parameter("yT", [DIM, S], BF16, isOutput=False)
    wq = nc.declare_dram_parameter("wq", [DIM, FS], BF16, isOutput=False)
    wkx = nc.declare_dram_parameter("wkx", [DIM, FS], BF16, isOutput=False)
    wky = nc.declare_dram_parameter("wky", [DIM, FS], BF16, isOutput=False)
    wvx = nc.declare_dram_parameter("wvx", [DIM, FS], BF16, isOutput=False)
    wvy = nc.declare_dram_parameter("wvy", [DIM, FS], BF16, isOutput=False)
    wo = nc.declare_dram_parameter("wo", [FS, DIM], BF16, isOutput=False)
    bq = nc.declare_dram_parameter("bq", [128, 4], FP32, isOutput=False)
    bkx = nc.declare_dram_parameter("bkx", [128, 4], FP32, isOutput=False)
    bky = nc.declare_dram_parameter("bky", [128, 4], FP32, isOutput=False)
    bvx_bc = nc.declare_dram_parameter("bvx_bc", [128, FS], FP32, isOutput=False)
    bvy_bc = nc.declare_dram_parameter("bvy_bc", [128, FS], FP32, isOutput=False)
    outT = nc.declare_dram_parameter("outT", [DIM, S], FP32, isOutput=True)

    EXP = mybir.ActivationFunctionType.Exp

    with tile.TileContext(nc) as tc:
        import contextlib

        with contextlib.ExitStack() as ctx:
            big = ctx.enter_context(tc.tile_pool(name="big", bufs=24))
            wpool = ctx.enter_context(tc.tile_pool(name="wpool", bufs=26))
            qkv = ctx.enter_context(tc.tile_pool(name="qkv", bufs=12))
            vpool = ctx.enter_context(tc.tile_pool(name="vpool", bufs=16))
            ppool = ctx.enter_context(tc.tile_pool(name="ppool", bufs=5))
            opool = ctx.enter_context(tc.tile_pool(name="opool", bufs=2))
            spool = ctx.enter_context(tc.tile_pool(name="spool", bufs=4))
            cpool = ctx.enter_context(tc.tile_pool(name="cpool", bufs=1))
            dpool = ctx.enter_context(tc.tile_pool(name="dpool", bufs=8, space="DRAM"))
            mm_ps = ctx.enter_context(tc.tile_pool(name="mm_ps", bufs=3, space="PSUM"))
            ot_ps = ctx.enter_context(tc.tile_pool(name="ot_ps", bufs=2, space="PSUM"))

            # ---- constants ----
            ones_f32 = cpool.tile([128, 64], FP32, tag="ones_f32")
            nc.vector.memset(ones_f32[:, :], 1.0)
            bq_sb = cpool.tile([128, 4], FP32, tag="bq")
            bkx_sb = cpool.tile([128, 4], FP32, tag="bkx")
            bky_sb = cpool.tile([128, 4], FP32, tag="bky")
            bvx_sb = cpool.tile([128, FS], FP32, tag="bvx")
            bvy_sb = cpool.tile([128, FS], FP32, tag="bvy")
            nc.sync.dma_start(out=bq_sb, in_=bq[:, :])
            nc.sync.dma_start(out=bkx_sb, in_=bkx[:, :])
            nc.sync.dma_start(out=bky_sb, in_=bky[:, :])
            nc.sync.dma_start(out=bvx_sb, in_=bvx_bc[:, :])
            nc.sync.dma_start(out=bvy_sb, in_=bvy_bc[:, :])

            # ---- load activations ----
            xt = []
            wvx_sb = []
            for i in range(8):
                t = big.tile([128, S], BF16, tag="big", name=f"xt{i}")
                nc.sync.dma_start(out=t, in_=xT[i * 128:(i + 1) * 128, :])
                xt.append(t)
                tw = wpool.tile([128, FS], BF16, tag="w", name=f"wvx{i}")
                nc.sync.dma_start(out=tw, in_=wvx[i * 128:(i + 1) * 128, :])
                wvx_sb.append(tw)
            yt = []

            # ---- V projections (natural domain, bias + ones column) ----
            V = [vpool.tile([128, HG, DH + 1], BF16, tag="v", name=f"V{i}") for i in range(16)]
            for src_is_y in (False, True):
                bias_sb = bvy_sb if src_is_y else bvx_sb
                base = 8 if src_is_y else 0
                if src_is_y:
                    w_sb = []
                    for i in range(8):
                        ty = big.tile([128, S], BF16, tag="big", name=f"yt{i}")
                        nc.sync.dma_start(out=ty, in_=yT[i * 128:(i + 1) * 128, :])
                        yt.append(ty)
                        tw = wpool.tile([128, FS], BF16, tag="w", name=f"wvy{i}")
                        nc.sync.dma_start(out=tw, in_=wvy[i * 128:(i + 1) * 128, :])
                        w_sb.append(tw)
                else:
                    w_sb = wvx_sb
                act = yt if src_is_y else xt
                for sg in range(4):  # two s-tiles per psum group
                    ps = mm_ps.tile([128, 1024], FP32, tag="mm", name="vps")
                    for ct in range(8):
                        for half in range(2):
                            st = 2 * sg + half
                            nc.tensor.matmul(
                                ps[:, half * 512:(half + 1) * 512],
                                xt[ct][:, st * 128:(st + 1) * 128] if not src_is_y
                                else yt[ct][:, st * 128:(st + 1) * 128],
                                w_sb[ct][:, :],
                                start=(ct == 0),
                                stop=(ct == 7),
                            )
                    for half in range(2):
                        st = 2 * sg + half
                        vt = V[base + st]
                        nc.vector.tensor_add(
                            out=vt[:, :, 0:DH],
                            in0=ps[:, half * 512:(half + 1) * 512].rearrange(
                                "p (h d) -> p h d", h=HG),
                            in1=bias_sb[:, :].rearrange("p (h d) -> p h d", h=HG),
                        )
                        nc.vector.tensor_copy(
                            out=vt[:, :, DH:DH + 1],
                            in_=ones_f32[:, 0:HG].rearrange("p (h o) -> p h o", o=1),
                        )

            # ---- Q/K projections (transposed domain [feat, seq]) ----
            QT = [qkv.tile([128, S], BF16, tag="qkv", name=f"QT{i}") for i in range(4)]
            KxT = [qkv.tile([128, S], BF16, tag="qkv", name=f"KxT{i}") for i in range(4)]
            KyT = [qkv.tile([128, S], BF16, tag="qkv", name=f"KyT{i}") for i in range(4)]

            qk_w = []
            for pi, w_dram in enumerate((wq, wkx, wky)):
                lst = []
                for ct in range(8):
                    tw = wpool.tile([128, FS], BF16, tag="w", name=f"wp{pi}_{ct}")
                    nc.sync.dma_start(
                        out=tw, in_=w_dram[ct * 128:(ct + 1) * 128, :]
                    )
                    lst.append(tw)
                qk_w.append(lst)
            qk_act = [xt, xt, yt]
            qk_bias = [bq_sb, bkx_sb, bky_sb]
            qk_dst = [QT, KxT, KyT]
            qk_ps = {}

            def emit_qk_half(pi, ft, half):
                key = (pi, ft)
                if key not in qk_ps:
                    qk_ps[key] = mm_ps.tile(
                        [128, 1024], FP32, tag="mm", name=f"qkps{pi}_{ft}"
                    )
                ps = qk_ps[key]
                w_sb = qk_w[pi]
                act = qk_act[pi]
                for ct in (range(0, 4) if half == 0 else range(4, 8)):
                    for h2 in range(2):
                        nc.tensor.matmul(
                            ps[:, h2 * 512:(h2 + 1) * 512],
                            w_sb[ct][:, ft * 128:(ft + 1) * 128],
                            act[ct][:, h2 * 512:(h2 + 1) * 512],
                            start=(ct == 0),
                            stop=(ct == 7),
                        )
                if half == 1:
                    nc.vector.tensor_scalar_add(
                        out=qk_dst[pi][ft][:, :],
                        in0=ps[:, :],
                        scalar1=qk_bias[pi][:, ft:ft + 1],
                    )
                    del qk_ps[key]

            # upfront: ft=0 for all projections, plus all fts of proj 2 (wky)
            for pi in range(3):
                emit_qk_half(pi, 0, 0)
                emit_qk_half(pi, 0, 1)
            for ft in range(1, 4):
                emit_qk_half(2, ft, 0)
                emit_qk_half(2, ft, 1)

            # remaining 6 groups ride the attention phase's spare PE cycles
            qk_fillers = {(t, qt): (qt, t + 1) for t in range(3) for qt in range(2)}

            # ---- prefetch out-projection weights during attention ----
            wo_sb = []
            for ft in range(4):
                two = big.tile([128, S], BF16, tag="big", name=f"wo{ft}")
                nc.sync.dma_start(out=two, in_=wo[ft * 128:(ft + 1) * 128, :])
                wo_sb.append(two)

            # ---- attention (head pairs row-pack; both q-tiles share one
            #      psum tile so exp runs once per (kt, head)) ----
            oT = [big.tile([128, S], BF16, tag="big", name=f"oT{i}") for i in range(4)]

            def emit_finalize(t, qt, o_sb, recips):
                for hh in range(2):
                        i = hh
                        rd = recips[i]
                        bc_sb = spool.tile([DH, 512], FP32, tag="bc", name="bc_sb")
                        rd_bcast = bass.AP(
                            tensor=rd.tensor, offset=rd.offset,
                            ap=[[0, DH]] + [list(a) for a in rd.ap[1:]],
                        )
                        nc.gpsimd.dma_start(out=bc_sb[:, :], in_=rd_bcast)
                        nc.vector.tensor_mul(
                            out=oT[t][hh * 64:hh * 64 + DH, qt * 512:(qt + 1) * 512],
                            in0=o_sb[i][0:DH, :],
                            in1=bc_sb[:, :],
                        )

            pending = None
            for t in range(4):  # heads 2t, 2t+1
                for qt in range(2):
                    o_ps = [ot_ps.tile([128, 512], FP32, tag="ot", name=f"ops{i}")
                            for i in range(2)]  # per head of the pair
                    prev = None
                    for kt in range(16):
                        KT = KxT[t] if kt < 8 else KyT[t]
                        ks = (kt % 8) * 128
                        sc = mm_ps.tile([128, 1024], FP32, tag="mm", name="sc")
                        for hh in range(2):
                            nc.tensor.matmul(
                                sc[:, hh * 512:(hh + 1) * 512],
                                KT[hh * 64:(hh + 1) * 64, ks:ks + 128],
                                QT[t][hh * 64:(hh + 1) * 64, qt * 512:(qt + 1) * 512],
                                start=True,
                                stop=True,
                            )
                        p2 = ppool.tile([128, 1024], BF16, tag="p", name="p")
                        nc.scalar.activation(out=p2[:, :], in_=sc[:, :], func=EXP)
                        if (t, qt) in qk_fillers and kt in (4, 11):
                            fpi, fft = qk_fillers[(t, qt)]
                            emit_qk_half(fpi, fft, 0 if kt == 4 else 1)
                        if prev is not None:
                            for hh in range(2):
                                nc.tensor.matmul(
                                    o_ps[hh][0:DH + 1, :],
                                    V[kt - 1][:, 2 * t + hh, :],
                                    prev[:, hh * 512:(hh + 1) * 512],
                                    start=(kt == 1),
                                    stop=False,
                                )
                        prev = p2
                    for hh in range(2):
                        nc.tensor.matmul(
                            o_ps[hh][0:DH + 1, :],
                            V[15][:, 2 * t + hh, :],
                            prev[:, hh * 512:(hh + 1) * 512],
                            start=False,
                            stop=True,
                        )
                    if pending is not None:
                        emit_finalize(*pending)
                    o_sb = []
                    recips = []
                    for i in range(2):
                        ob = spool.tile([DH + 1, 512], FP32, tag="osb", name="osb")
                        nc.vector.tensor_copy(out=ob[:, :], in_=o_ps[i][0:DH + 1, :])
                        o_sb.append(ob)
                    for i in range(2):
                        rf = spool.tile([1, 512], FP32, tag="recipf", name="rf")
                        nc.vector.reciprocal(out=rf[:, :], in_=o_sb[i][DH:DH + 1, :])
                        rd = dpool.tile([1, 512], FP32, name="rd")
                        nc.gpsimd.dma_start(out=rd[:, :], in_=rf[:, :])
                        recips.append(rd)
                    pending = (t, qt, o_sb, recips)
            emit_finalize(*pending)

            # ---- out-projection (transposed domain [m, s]) ----
            for mt in range(8):
                ps = mm_ps.tile([128, 1024], FP32, tag="mm", name="obs")
                for ft in range(4):
                    for half in range(2):
                        nc.tensor.matmul(
                            ps[:, half * 512:(half + 1) * 512],
                            wo_sb[ft][:, mt * 128:(mt + 1) * 128],
                            oT[ft][:, half * 512:(half + 1) * 512],
                            start=(ft == 0),
                            stop=(ft == 3),
                        )
                osb = opool.tile([128, 1024], FP32, tag="osb2", name="osb2")
                nc.vector.tensor_copy(out=osb[:, :], in_=ps[:, :])
                nc.sync.dma_start(
                    out=outT[mt * 128:(mt + 1) * 128, :],
                    in_=osb[:, :],
                )

    _spill_excess_waits(nc)
    return nc


_NC = None


def _get_program():
    global _NC
    if _NC is None:
        _NC = _build_program()
    return _NC


# ---------------------------------------------------------------------------
# host wrapper
# ---------------------------------------------------------------------------

def _prep_in_maps(x, y, W_Kx, b_Kx, W_Qx, b_Qx, W_Vx, b_Vx, W_Ky, b_Ky,
                  W_Vy, b_Vy, W_out, b_out):
    f32 = np.float32
    bf16 = ml_dtypes.bfloat16
    in_maps = []
    for c in range(NCORES):
        b = c // 2
        g = c % 2
        gs = slice(FS * g, FS * (g + 1))
        m = {
            "xT": np.ascontiguousarray(np.asarray(x[b], f32).T).astype(bf16),
            "yT": np.ascontiguousarray(np.asarray(y[b], f32).T).astype(bf16),
            "wq": np.ascontiguousarray((np.asarray(W_Qx, f32)[gs, :] / 8.0).T).astype(bf16),
            "wkx": np.ascontiguousarray(np.asarray(W_Kx, f32)[gs, :].T).astype(bf16),
            "wky": np.ascontiguousarray(np.asarray(W_Ky, f32)[gs, :].T).astype(bf16),
            "wvx": np.ascontiguousarray(np.asarray(W_Vx, f32)[gs, :].T).astype(bf16),
            "wvy": np.ascontiguousarray(np.asarray(W_Vy, f32)[gs, :].T).astype(bf16),
            "wo": np.ascontiguousarray(np.asarray(W_out, f32)[:, gs].T).astype(bf16),
            "bq": np.ascontiguousarray(
                (np.asarray(b_Qx, f32)[gs] / 8.0).reshape(4, 128).T),
            "bkx": np.ascontiguousarray(np.asarray(b_Kx, f32)[gs].reshape(4, 128).T),
            "bky": np.ascontiguousarray(np.asarray(b_Ky, f32)[gs].reshape(4, 128).T),
            "bvx_bc": np.ascontiguousarray(
                np.broadcast_to(np.asarray(b_Vx, f32)[gs], (128, FS))),
            "bvy_bc": np.ascontiguousarray(
                np.broadcast_to(np.asarray(b_Vy, f32)[gs], (128, FS))),
        }
        in_maps.append(m)
    return in_maps


def _assemble(results, b_out):
    B = 4
    out = np.empty((B, S, DIM), np.float32)
    bo = np.asarray(b_out, np.float32)
    for b in range(B):
        acc = results[2 * b]["outT"] + results[2 * b + 1]["outT"]
        out[b] = acc.T + bo
    return out


def kernel(**inputs):
    nc = _get_program()
    in_maps = _prep_in_maps(**inputs)
    last_err = None
    for _attempt in range(3):
        try:
            res = run_bass_kernel_spmd(nc, in_maps, core_ids=list(range(NCORES)))
            return _assemble(res.results, inputs["b_out"])
        except Exception as e:  # transient NRT_EXEC_UNIT_UNRECOVERABLE after fresh compile
            last_err = e
            import time as _time
            _time.sleep(2.0)
    raise last_err


def kernel_traced(trace_cores=None, **inputs):
    """Same as kernel() but returns (out, BassKernelResults) with NTFF trace."""
    _register_ntff_hook()
    nc = _get_program()
    in_maps = _prep_in_maps(**inputs)
    res = run_bass_kernel_spmd(
        nc, in_maps, core_ids=list(range(NCORES)), trace=True,
        trace_cores=trace_cores or [0],
    )
    return _assemble(res.results, inputs["b_out"]), res


# revision 23
# speedup vs baseline: 1.6361x; 1.0127x over previous
"""MultiHeadCrossAttention kernel for 8 Trainium2 NeuronCores.

Problem (hardcoded): B=4, Sx=Sy=1024, DIM=1024, H=16, Dh=64, fp32.
  Q = x@W_Qx.T+b_Qx ; K = cat(x@W_Kx.T+b_Kx, y@W_Ky.T+b_Ky) per head
  V = cat(x@W_Vx.T+b_Vx, y@W_Vy.T+b_Vy) ; out = softmax(QK^T/8)V @ W_out.T + b_out

Sharding: core c -> (batch b = c//2, head-group g = c%2 of 8 heads).
Each core computes its batch's attention for its 8 heads plus the partial
out-projection over its 512 features; host sums the two partials per batch
and adds b_out (the "all-reduce after to_out", done in the gather).

Device layout choices (all matmuls natural, zero on-device transposes):
 - activations pre-transposed on host: xT/yT [dim, seq]
 - Q/K projections in transposed domain [feat, seq]  (bias = per-partition)
 - V in natural domain [seq, feat] with host-broadcast bias, plus a ones
   column per head -> AV matmul row 64 yields the softmax denominator
 - scoresT [k, q] via lhsT=KT (d=64 contraction; head pairs row-pack the PE)
 - exp on ACT only (no max subtraction: |scores| <~ 3), normalize via
   PE-broadcast reciprocal, out-projection in transposed domain [m, s]
 - float32r everywhere on the PE: full rate at N=512, ~5e-5 rel err
"""

import os
import sys

os.environ.setdefault("MYCRO_LOCAL_CACHE", "1")
if "/opt/trn_rl_repo" not in sys.path:
    sys.path.insert(0, "/opt/trn_rl_repo")

import ml_dtypes
import numpy as np

import concourse.bass as bass
import concourse.mybir as mybir
import concourse.tile as tile
from concourse import bass_utils
from concourse.bass_utils import run_bass_kernel_spmd

FP32 = mybir.dt.float32
FP32R = mybir.dt.float32r
BF16 = mybir.dt.bfloat16

DIM = 1024
H = 16          # total heads
HG = 8          # heads per core (head-group)
DH = 64
S = 1024        # Sx = Sy
FS = 512        # feature slice per core (HG * DH)
NCORES = 8

# ---------------------------------------------------------------------------
# harness patches (this snapshot's Tile emits >1 wait per instruction in a
# few places; HW instructions hold one wait)
# ---------------------------------------------------------------------------

def _patched_drain_and_barrier(self, tick_clock, wait_clock):
    from bass_rust import ScopedClock

    nc = self.nc
    drain_inst = nc.sync.drain()
    wait_clock.add_sem_waits(
        drain_inst.ins, ScopedClock({None: tick_clock.global_clock})
    )
    si = drain_inst.ins.sync_info
    waits = list(si.on_wait)
    if len(waits) > 1:
        del si.on_wait[1:]
        for w in waits[1:]:
            nop = nc.sync.nop(nofuse=True, hint="drain_wait_spill")
            if nop.ins.sync_info is None:
                nop.ins.sync_info = mybir.SyncInfo(on_wait=[], on_update=[])
            nop.ins.sync_info.on_wait.append(w)

    nc.all_engine_barrier()
    assert self.sems is not None
    popped = nc._tile_sem_poison_stack.pop()
    assert popped is self._sem_poison
    nc.clear_and_free_semaphores(list(self.sems.allocated().values()))
    nc.all_engine_barrier()


def _spill_excess_waits(nc):
    n = 0
    for fn in nc.m.functions:
        for bb in fn.blocks:
            new_insts = []
            for inst in bb.instructions:
                si = getattr(inst, "sync_info", None)
                cap = 2 if isinstance(inst, mybir.InstEventSemaphore) else 1
                if si is not None and si.on_wait and len(si.on_wait) > cap:
                    extras = list(si.on_wait[cap:])
                    del si.on_wait[cap:]
                    for w in extras:
                        new_insts.append(
                            mybir.InstNoOp(
                                name=f"wspill-{nc.next_id()}",
                                engine=inst.engine,
                                ins=[],
                                outs=[],
                                sync_info=mybir.SyncInfo(on_wait=[w], on_update=[]),
                            )
                        )
                        n += 1
                new_insts.append(inst)
            bb.instructions[:] = new_insts
    return n


tile.TileContext._drain_and_barrier = _patched_drain_and_barrier

if os.environ.get("ENABLE_LDW_OPT") == "1":
    _orig_run_command = bass_utils.run_command

    def _run_command_ldw(argv, **kwargs):
        if isinstance(argv, list):
            argv = ["--enable-ldw-opt=true" if a == "--enable-ldw-opt=false" else a
                    for a in argv]
        return _orig_run_command(argv, **kwargs)

    bass_utils.run_command = _run_command_ldw
bass_utils.upload_artifacts = lambda tmpdir: tmpdir  # no S3 in container


def _register_ntff_hook():
    """Best-effort: enables trace=True runs (used by test harness only)."""
    try:
        from antenv.axon_hooks import set_axon_ntff_profile_hook
        sys.path.insert(0, "/root/.axon_site")
        from trn_agent_boot.trn_boot import _ntff_profile_via_ctypes

        set_axon_ntff_profile_hook(
            _ntff_profile_via_ctypes("/opt/axon/libaxon_pjrt.so")
        )
    except Exception:
        pass


# ---------------------------------------------------------------------------
# device program (identical on all 8 cores; per-core data differs)
# ---------------------------------------------------------------------------

def _build_program():
    nc = bass.Bass()

    xT = nc.declare_dram_parameter("xT", [DIM, S], BF16, isOutput=False)
    yT = nc.declare_dram_parameter("yT", [DIM, S], BF16, isOutput=False)
    wq = nc.declare_dram_parameter("wq", [DIM, FS], BF16, isOutput=False)
    wkx = nc.declare_dram_parameter("wkx", [DIM, FS], BF16, isOutput=False)
    wky = nc.declare_dram_parameter("wky", [DIM, FS], BF16, isOutput=False)
    wvx = nc.declare_dram_parameter("wvx", [DIM, FS], BF16, isOutput=False)
    wvy = nc.declare_dram_parameter("wvy", [DIM, FS], BF16, isOutput=False)
    wo = nc.declare_dram_parameter("wo", [FS, DIM], BF16, isOutput=False)
    bq = nc.declare_dram_parameter("bq", [128, 4], FP32, isOutput=False)
    bkx = nc.declare_dram_parameter("bkx", [128, 4], FP32, isOutput=False)
    bky = nc.declare_dram_parameter("bky", [128, 4], FP32, isOutput=False)
    bvx_bc = nc.declare_dram_parameter("bvx_bc", [128, FS], FP32, isOutput=False)
    bvy_bc = nc.declare_dram_parameter("bvy_bc", [128, FS], FP32, isOutput=False)
    outT = nc.declare_dram_parameter("outT", [DIM, S], FP32, isOutput=True)

    EXP = mybir.ActivationFunctionType.Exp

    with tile.TileContext(nc) as tc:
        import contextlib

        with contextlib.ExitStack() as ctx:
            big = ctx.enter_context(tc.tile_pool(name="big", bufs=24))
            wpool = ctx.enter_context(tc.tile_pool(name="wpool", bufs=26))
            qkv = ctx.enter_context(tc.tile_pool(name="qkv", bufs=12))
            vpool = ctx.enter_context(tc.tile_pool(name="vpool", bufs=16))
            ppool = ctx.enter_context(tc.tile_pool(name="ppool", bufs=5))
            opool = ctx.enter_context(tc.tile_pool(name="opool", bufs=2))
            spool = ctx.enter_context(tc.tile_pool(name="spool", bufs=4))
            cpool = ctx.enter_context(tc.tile_pool(name="cpool", bufs=1))
            dpool = ctx.enter_context(tc.tile_pool(name="dpool", bufs=8, space="DRAM"))
            mm_ps = ctx.enter_context(tc.tile_pool(name="mm_ps", bufs=3, space="PSUM"))
            ot_ps = ctx.enter_context(tc.tile_pool(name="ot_ps", bufs=2, space="PSUM"))

            # ---- constants ----
            ones_f32 = cpool.tile([128, 64], FP32, tag="ones_f32")
            nc.vector.memset(ones_f32[:, :], 1.0)
            bq_sb = cpool.tile([128, 4], FP32, tag="bq")
            bkx_sb = cpool.tile([128, 4], FP32, tag="bkx")
            bky_sb = cpool.tile([128, 4], FP32, tag="bky")
            bvx_sb = cpool.tile([128, FS], FP32, tag="bvx")
            bvy_sb = cpool.tile([128, FS], FP32, tag="bvy")
            nc.sync.dma_start(out=bq_sb, in_=bq[:, :])
            nc.sync.dma_start(out=bkx_sb, in_=bkx[:, :])
            nc.sync.dma_start(out=bky_sb, in_=bky[:, :])
            nc.sync.dma_start(out=bvx_sb, in_=bvx_bc[:, :])
            nc.sync.dma_start(out=bvy_sb, in_=bvy_bc[:, :])

            # ---- load activations ----
            xt = []
            wvx_sb = []
            for i in range(8):
                t = big.tile([128, S], BF16, tag="big", name=f"xt{i}")
                nc.sync.dma_start(out=t, in_=xT[i * 128:(i + 1) * 128, :])
                xt.append(t)
                tw = wpool.tile([128, FS], BF16, tag="w", name=f"wvx{i}")
                nc.sync.dma_start(out=tw, in_=wvx[i * 128:(i + 1) * 128, :])
                wvx_sb.append(tw)
            yt = []

            # ---- V projections (natural domain, bias + ones column) ----
            V = [vpool.tile([128, HG, DH + 1], BF16, tag="v", name=f"V{i}") for i in range(16)]
            for src_is_y in (False, True):
                bias_sb = bvy_sb if src_is_y else bvx_sb
                base = 8 if src_is_y else 0
                if src_is_y:
                    w_sb = []
                    for i in range(8):
                        ty = big.tile([128, S], BF16, tag="big", name=f"yt{i}")
                        nc.sync.dma_start(out=ty, in_=yT[i * 128:(i + 1) * 128, :])
                        yt.append(ty)
                        tw = wpool.tile([128, FS], BF16, tag="w", name=f"wvy{i}")
                        nc.sync.dma_start(out=tw, in_=wvy[i * 128:(i + 1) * 128, :])
                        w_sb.append(tw)
                else:
                    w_sb = wvx_sb
                act = yt if src_is_y else xt
                for sg in range(4):  # two s-tiles per psum group
                    ps = mm_ps.tile([128, 1024], FP32, tag="mm", name="vps")
                    for ct in range(8):
                        for half in range(2):
                            st = 2 * sg + half
                            nc.tensor.matmul(
                                ps[:, half * 512:(half + 1) * 512],
                                xt[ct][:, st * 128:(st + 1) * 128] if not src_is_y
                                else yt[ct][:, st * 128:(st + 1) * 128],
                                w_sb[ct][:, :],
                                start=(ct == 0),
                                stop=(ct == 7),
                            )
                    for half in range(2):
                        st = 2 * sg + half
                        vt = V[base + st]
                        nc.vector.tensor_add(
                            out=vt[:, :, 0:DH],
                            in0=ps[:, half * 512:(half + 1) * 512].rearrange(
                                "p (h d) -> p h d", h=HG),
                            in1=bias_sb[:, :].rearrange("p (h d) -> p h d", h=HG),
                        )
                        nc.vector.tensor_copy(
                            out=vt[:, :, DH:DH + 1],
                            in_=ones_f32[:, 0:HG].rearrange("p (h o) -> p h o", o=1),
                        )

            # ---- Q/K projections (transposed domain [feat, seq]) ----
            QT = [qkv.tile([128, S], BF16, tag="qkv", name=f"QT{i}") for i in range(4)]
            KxT = [qkv.tile([128, S], BF16, tag="qkv", name=f"KxT{i}") for i in range(4)]
            KyT = [qkv.tile([128, S], BF16, tag="qkv", name=f"KyT{i}") for i in range(4)]

            qk_w = []
            for pi, w_dram in enumerate((wq, wkx, wky)):
                lst = []
                for ct in range(8):
                    tw = wpool.tile([128, FS], BF16, tag="w", name=f"wp{pi}_{ct}")
                    nc.sync.dma_start(
                        out=tw, in_=w_dram[ct * 128:(ct + 1) * 128, :]
                    )
                    lst.append(tw)
                qk_w.append(lst)
            qk_act = [xt, xt, yt]
            qk_bias = [bq_sb, bkx_sb, bky_sb]
            qk_dst = [QT, KxT, KyT]
            qk_ps = {}

            def emit_qk_half(pi, ft, half):
                key = (pi, ft)
                if key not in qk_ps:
                    qk_ps[key] = mm_ps.tile(
                        [128, 1024], FP32, tag="mm", name=f"qkps{pi}_{ft}"
                    )
                ps = qk_ps[key]
                w_sb = qk_w[pi]
                act = qk_act[pi]
                for ct in (range(0, 4) if half == 0 else range(4, 8)):
                    for h2 in range(2):
                        nc.tensor.matmul(
                            ps[:, h2 * 512:(h2 + 1) * 512],
                            w_sb[ct][:, ft * 128:(ft + 1) * 128],
                            act[ct][:, h2 * 512:(h2 + 1) * 512],
                            start=(ct == 0),
                            stop=(ct == 7),
                        )
                if half == 1:
                    nc.vector.tensor_scalar_add(
                        out=qk_dst[pi][ft][:, :],
                        in0=ps[:, :],
                        scalar1=qk_bias[pi][:, ft:ft + 1],
                    )
                    del qk_ps[key]

            # upfront: ft=0 for all projections, plus all fts of proj 2 (wky)
            for pi in range(3):
                emit_qk_half(pi, 0, 0)
                emit_qk_half(pi, 0, 1)
            for ft in range(1, 4):
                emit_qk_half(2, ft, 0)
                emit_qk_half(2, ft, 1)

            # remaining 6 groups ride the attention phase's spare PE cycles
            qk_fillers = {(t, qt): (qt, t + 1) for t in range(3) for qt in range(2)}

            # ---- prefetch out-projection weights during attention ----
            wo_sb = []
            for ft in range(4):
                two = big.tile([128, S], BF16, tag="big", name=f"wo{ft}")
                nc.sync.dma_start(out=two, in_=wo[ft * 128:(ft + 1) * 128, :])
                wo_sb.append(two)

            # ---- attention (head pairs row-pack; both q-tiles share one
            #      psum tile so exp runs once per (kt, head)) ----
            oT = [big.tile([128, S], BF16, tag="big", name=f"oT{i}") for i in range(4)]

            def emit_finalize(t, qt, o_sb, recips):
                for hh in range(2):
                        i = hh
                        rd = recips[i]
                        bc_sb = spool.tile([DH, 512], FP32, tag="bc", name="bc_sb")
                        rd_bcast = bass.AP(
                            tensor=rd.tensor, offset=rd.offset,
                            ap=[[0, DH]] + [list(a) for a in rd.ap[1:]],
                        )
                        nc.gpsimd.dma_start(out=bc_sb[:, :], in_=rd_bcast)
                        nc.vector.tensor_mul(
                            out=oT[t][hh * 64:hh * 64 + DH, qt * 512:(qt + 1) * 512],
                            in0=o_sb[i][:, :],
                            in1=bc_sb[:, :],
                        )

            pending = None
            for t in range(4):  # heads 2t, 2t+1
                for qt in range(2):
                    o_ps = [ot_ps.tile([128, 512], FP32, tag="ot", name=f"ops{i}")
                            for i in range(2)]  # per head of the pair
                    prev = None
                    for kt in range(16):
                        KT = KxT[t] if kt < 8 else KyT[t]
                        ks = (kt % 8) * 128
                        sc = mm_ps.tile([128, 1024], FP32, tag="mm", name="sc")
                        for hh in range(2):
                            nc.tensor.matmul(
                                sc[:, hh * 512:(hh + 1) * 512],
                                KT[hh * 64:(hh + 1) * 64, ks:ks + 128],
                                QT[t][hh * 64:(hh + 1) * 64, qt * 512:(qt + 1) * 512],
                                start=True,
                                stop=True,
                            )
                        p2 = ppool.tile([128, 1024], BF16, tag="p", name="p")
                        nc.scalar.activation(out=p2[:, :], in_=sc[:, :], func=EXP)
                        if (t, qt) in qk_fillers and kt in (4, 11):
                            fpi, fft = qk_fillers[(t, qt)]
                            emit_qk_half(fpi, fft, 0 if kt == 4 else 1)
                        if prev is not None:
                            for hh in range(2):
                                nc.tensor.matmul(
                                    o_ps[hh][0:DH + 1, :],
                                    V[kt - 1][:, 2 * t + hh, :],
                                    prev[:, hh * 512:(hh + 1) * 512],
                                    start=(kt == 1),
                                    stop=False,
                                )
                        prev = p2
                    for hh in range(2):
                        nc.tensor.matmul(
                            o_ps[hh][0:DH + 1, :],
                            V[15][:, 2 * t + hh, :],
                            prev[:, hh * 512:(hh + 1) * 512],
                            start=False,
                            stop=True,
                        )
                    if pending is not None:
                        emit_finalize(*pending)
                    o_sb = []
                    s2 = spool.tile([33, 512], FP32, tag="s2", name="s2")
                    for i in range(2):
                        nc.vector.tensor_copy(
                            out=s2[32 * i:32 * i + 1, :], in_=o_ps[i][DH:DH + 1, :]
                        )
                        ob = spool.tile([DH, 512], FP32, tag="osb", name="osb")
                        nc.vector.tensor_copy(out=ob[:, :], in_=o_ps[i][0:DH, :])
                        o_sb.append(ob)
                    rf2 = spool.tile([33, 512], FP32, tag="recipf", name="rf2")
                    nc.vector.reciprocal(out=rf2[:, :], in_=s2[:, :])
                    recips = []
                    for i in range(2):
                        rd = dpool.tile([1, 512], FP32, name="rd")
                        nc.gpsimd.dma_start(out=rd[:, :], in_=rf2[32 * i:32 * i + 1, :])
                        recips.append(rd)
                    pending = (t, qt, o_sb, recips)
            emit_finalize(*pending)

            # ---- out-projection (transposed domain [m, s]) ----
            def op_mms(ps, mt, fts):
                for ft in fts:
                    for half in range(2):
                        nc.tensor.matmul(
                            ps[:, half * 512:(half + 1) * 512],
                            wo_sb[ft][:, mt * 128:(mt + 1) * 128],
                            oT[ft][:, half * 512:(half + 1) * 512],
                            start=(ft == 0),
                            stop=(ft == 3),
                        )

            def op_finish(ps, mt):
                osb = opool.tile([128, 1024], FP32, tag="osb2", name="osb2")
                nc.vector.tensor_copy(out=osb[:, :], in_=ps[:, :])
                nc.sync.dma_start(
                    out=outT[mt * 128:(mt + 1) * 128, :],
                    in_=osb[:, :],
                )

            wave = [mm_ps.tile([128, 1024], FP32, tag="mm", name=f"obs{m}")
                    for m in range(3)]
            for m in range(3):
                op_mms(wave[m], m, range(3))  # ft0-2: independent of last finalize
            for m in range(3):
                op_mms(wave[m], m, [3])
                op_finish(wave[m], m)
            for mt in range(3, 8):
                ps = mm_ps.tile([128, 1024], FP32, tag="mm", name="obs")
                op_mms(ps, mt, range(4))
                op_finish(ps, mt)

    _spill_excess_waits(nc)
    return nc


_NC = None


def _get_program():
    global _NC
    if _NC is None:
        _NC = _build_program()
    return _NC


# ---------------------------------------------------------------------------
# host wrapper
# ---------------------------------------------------------------------------

def _prep_in_maps(x, y, W_Kx, b_Kx, W_Qx, b_Qx, W_Vx, b_Vx, W_Ky, b_Ky,
                  W_Vy, b_Vy, W_out, b_out):
    f32 = np.float32
    bf16 = ml_dtypes.bfloat16
    in_maps = []
    for c in range(NCORES):
        b = c // 2
        g = c % 2
        gs = slice(FS * g, FS * (g + 1))
        m = {
            "xT": np.ascontiguousarray(np.asarray(x[b], f32).T).astype(bf16),
            "yT": np.ascontiguousarray(np.asarray(y[b], f32).T).astype(bf16),
            "wq": np.ascontiguousarray((np.asarray(W_Qx, f32)[gs, :] / 8.0).T).astype(bf16),
            "wkx": np.ascontiguousarray(np.asarray(W_Kx, f32)[gs, :].T).astype(bf16),
            "wky": np.ascontiguousarray(np.asarray(W_Ky, f32)[gs, :].T).astype(bf16),
            "wvx": np.ascontiguousarray(np.asarray(W_Vx, f32)[gs, :].T).astype(bf16),
            "wvy": np.ascontiguousarray(np.asarray(W_Vy, f32)[gs, :].T).astype(bf16),
            "wo": np.ascontiguousarray(np.asarray(W_out, f32)[:, gs].T).astype(bf16),
            "bq": np.ascontiguousarray(
                (np.asarray(b_Qx, f32)[gs] / 8.0).reshape(4, 128).T),
            "bkx": np.ascontiguousarray(np.asarray(b_Kx, f32)[gs].reshape(4, 128).T),
            "bky": np.ascontiguousarray(np.asarray(b_Ky, f32)[gs].reshape(4, 128).T),
            "bvx_bc": np.ascontiguousarray(
                np.broadcast_to(np.asarray(b_Vx, f32)[gs], (128, FS))),
            "bvy_bc": np.ascontiguousarray(
                np.broadcast_to(np.asarray(b_Vy, f32)[gs], (128, FS))),
        }
        in_maps.append(m)
    return in_maps


def _assemble(results, b_out):
    B = 4
    out = np.empty((B, S, DIM), np.float32)
    bo = np.asarray(b_out, np.float32)
    for b in range(B):
        acc = results[2 * b]["outT"] + results[2 * b + 1]["outT"]
        out[b] = acc.T + bo
    return out


def kernel(**inputs):
    nc = _get_program()
    in_maps = _prep_in_maps(**inputs)
    last_err = None
    for _attempt in range(3):
        try:
            res = run_bass_kernel_spmd(nc, in_maps, core_ids=list(range(NCORES)))
            return _assemble(res.results, inputs["b_out"])
        except Exception as e:  # transient NRT_EXEC_UNIT_UNRECOVERABLE after fresh compile
            last_err = e
            import time as _time
            _time.sleep(2.0)
    raise last_err


def kernel_traced(trace_cores=None, **inputs):
    """Same as kernel() but returns (out, BassKernelResults) with NTFF trace."""
    _register_ntff_hook()
    nc = _get_program()
    in_maps = _prep_in_maps(**inputs)
    res = run_bass_kernel_spmd(
        nc, in_maps, core_ids=list(range(NCORES)), trace=True,
        trace_cores=trace_cores or [0],
    )
    return _assemble(res.results, inputs["b_out"]), res
